# revision 23
# baseline (speedup 1.0000x reference)
"""CoHHN GNN message-passing kernel for 8 Trainium2 NeuronCores.

Strategy (sharding_hint): shard the n_node dimension across the 8 cores.
 - adjacency COO entries partitioned by destination row; within a core the
   entries are grouped into 128-row windows, gathered from the (replicated /
   all-gathered) bf16 node table via dma_gather (one row -> one partition),
   and scatter-added through a one-hot matmul (one-hot built by
   gpsimd.local_scatter, values folded into the one-hot).
 - price/category tables + gate weights replicated; the big intra_gate
   contractions over n_node are sharded and finished with a small AllReduce.
 - item embeddings are all-gathered (bf16) between layers for the column
   gather in spmm / intra_gate.
All matmuls run in bf16 with fp32 PSUM accumulation; everything else fp32.
"""
import os
import sys
import math
import numpy as np
import ml_dtypes

sys.path.insert(0, '/opt/trn_rl_repo')

BF16 = ml_dtypes.bfloat16
P = 128          # partitions / EMB
EMB = 128


class Cfg:
    def __init__(self, n_node=50000, n_price=100, n_cat=500, ncore=8,
                 loh=32768, sg=12, gch=4):
        self.N = n_node
        self.NP = n_price
        self.NCAT = n_cat
        self.NCORE = ncore
        self.LOH = loh            # int16 gather-index limit (lo/hi table split)
        self.SG = sg              # blocks per local_scatter call (SG*128 <= 2047)
        self.GCH = gch            # blocks per dma_gather call
        self.NSH = n_node // ncore              # rows per core (assume divisible)
        assert self.NSH * ncore == n_node
        self.JT = math.ceil(self.NSH / P)       # 128-row j-tiles per shard
        self.NSHP = self.JT * P                 # padded shard rows (table layout)
        self.NTN = math.ceil(self.NSH / 512)    # node tiles of 512
        self.NPAD = self.NTN * 512              # padded node axis
        self.NW = self.NPAD // P                # adj windows (128 rows)
        self.NWR = self.JT                      # real windows (rest all-pad)
        self.NCATP = math.ceil(self.NCAT / P) * P
        self.NQC = self.NCATP // P              # cate chunks
        self.NTABP = self.NSHP * ncore          # padded global table rows


def _bf(x):
    return np.asarray(x, np.float32).astype(BF16)


def _wrap_idx(idx):
    """int16 stream -> dma_gather layout (128, len/16): slot i at [i%16, i//16],
    replicated across the 8 q7 cores."""
    idx = np.asarray(idx, np.int16)
    assert len(idx) % 16 == 0
    w = idx.reshape(-1, 16).T          # (16, L/16)
    return np.tile(w, (8, 1)).copy()   # (128, L/16)


def prep(inputs, cfg: Cfg):
    """Host-side sharding/layout prep. Only index manipulation, dtype casts,
    transposes and replication happen here - all arithmetic is on-device."""
    c = cfg
    f32 = np.float32
    emb = np.asarray(inputs['embedding'], f32)
    pri = np.asarray(inputs['pri_emb'], f32)
    cate = np.asarray(inputs['cate_emb'], f32)

    # -------- price/cate row permutations (positive mat_v first) --------
    m_pv = np.asarray(inputs['mat_pv'], f32)[:, 0]
    m_cv = np.asarray(inputs['mat_cv'], f32)[:, 0]
    perm_p = np.argsort(m_pv <= 0, kind='stable')
    perm_c = np.argsort(m_cv <= 0, kind='stable')
    npos_p = int((m_pv > 0).sum())
    npos_c = int((m_cv > 0).sum())
    inv_p = np.empty(c.NP, np.int64); inv_p[perm_p] = np.arange(c.NP)
    inv_c = np.empty(c.NCAT, np.int64); inv_c[perm_c] = np.arange(c.NCAT)

    m_pc = np.asarray(inputs['mat_pc'], f32)[:, 0][perm_p]   # per price row
    m_cp = np.asarray(inputs['mat_cp'], f32)[:, 0][perm_c]   # per cate row
    m_pv_s = m_pv[perm_p]
    m_cv_s = m_cv[perm_c]

    pri_s = pri[perm_p]
    cate_s = cate[perm_c]

    # -------- node table: padded layout (core section = NSHP rows) --------
    def pad_rows(a, rows):
        out = np.zeros((rows, a.shape[1]), a.dtype)
        out[:a.shape[0]] = a
        return out

    emb_tab = np.zeros((c.NTABP, EMB), BF16)
    for co in range(c.NCORE):
        emb_tab[co * c.NSHP: co * c.NSHP + c.NSH] = _bf(emb[co * c.NSH:(co + 1) * c.NSH])

    def padid(r):
        return (r // c.NSH) * c.NSHP + (r % c.NSH)

    # -------- adjacency: partition by destination row, window-group --------
    rows = np.asarray(inputs['adj_rows'], np.int64)
    cols = padid(np.asarray(inputs['adj_cols'], np.int64))
    vals = np.asarray(inputs['adj_vals'], f32)
    core_of = rows // c.NSH
    lr = rows % c.NSH
    win = lr // P
    wrow = lr % P
    is_lo = cols < c.LOH

    # counts per (core, window, side)
    cnt = np.zeros((c.NCORE, c.NW, 2), np.int64)
    np.add.at(cnt, (core_of, win, np.where(is_lo, 0, 1)), 1)
    nlo = np.ceil(cnt[:, :, 0].max(axis=0) / P).astype(np.int64)
    nhi = np.ceil(cnt[:, :, 1].max(axis=0) / P).astype(np.int64)

    # B-order block list
    blocks = []  # (w, side, slot_in_side_stream)
    locnt = hicnt = 0
    for w in range(c.NW):
        for _ in range(nlo[w]):
            blocks.append((w, 0, locnt)); locnt += 1
        for _ in range(nhi[w]):
            blocks.append((w, 1, hicnt)); hicnt += 1
    nblk_real = len(blocks)

    # scatter calls: SG-block groups, padded to even block count with dummies
    blocks_p = []
    scalls = []   # (k0, nb)
    for s in range(0, nblk_real, c.SG):
        grp = list(blocks[s:s + c.SG])
        if len(grp) % 2:
            grp.append((None, None, None))
        scalls.append((len(blocks_p), len(grp)))
        blocks_p.extend(grp)
    nblkp = len(blocks_p)

    glo_calls = [(s, min(c.GCH, locnt - s)) for s in range(0, locnt, c.GCH)]
    ghi_calls = [(s, min(c.GCH, hicnt - s)) for s in range(0, hicnt, c.GCH)]

    # per-core streams
    per_core = []
    order = np.lexsort((wrow, win, core_of))
    cols_o, vals_o, core_o, win_o, wrow_o, islo_o = (
        cols[order], vals[order], core_of[order], win[order], wrow[order], is_lo[order])
    for co in range(c.NCORE):
        sel = core_o == co
        cw, vv, ww, wr, lo = cols_o[sel], vals_o[sel], win_o[sel], wrow_o[sel], islo_o[sel]
        lo_idx = np.zeros(locnt * P, np.int16)
        hi_idx = np.zeros(hicnt * P, np.int16)
        s_idx = np.full((P, nblkp), -1, np.int16)
        s_val = np.zeros((P, nblkp), BF16)
        locur = hicur = 0
        for w in range(c.NW):
            wm = ww == w
            for side, n_side, cur in ((0, nlo[w], locur), (1, nhi[w], hicur)):
                sm = wm & (lo if side == 0 else ~lo)
                cw_s, vv_s, wr_s = cw[sm], vv[sm], wr[sm]
                ne = len(cw_s)
                assert ne <= n_side * P
                stream = lo_idx if side == 0 else hi_idx
                base = cur * P
                stream[base:base + ne] = (cw_s - (0 if side == 0 else c.LOH)).astype(np.int16)
                if side == 0:
                    locur += n_side
                else:
                    hicur += n_side
        # fill scatter idx/val arrays walking padded blocks
        # (entry q of (w,side) stream maps to block slot q//128, partition q%128)
        # rebuild per (w, side) entry arrays:
        ptr = {}
        for w in range(c.NW):
            wm = ww == w
            for side in (0, 1):
                sm = wm & (lo if side == 0 else ~lo)
                ptr[(w, side)] = (vv[sm], wr[sm])
        side_pos = {}
        for kp, (w, side, slot) in enumerate(blocks_p):
            if w is None:
                continue
            vv_s, wr_s = ptr[(w, side)]
            j = side_pos.get((w, side), 0)
            lo_e = j * P
            hi_e = min(lo_e + P, len(vv_s))
            n = hi_e - lo_e
            if n > 0:
                call_rel = kp - [s for s, nb in scalls if s <= kp < s + nb][0]
                s_idx[:n, kp] = (call_rel * P + wr_s[lo_e:hi_e]).astype(np.int16)
                s_val[:n, kp] = vv_s[lo_e:hi_e].astype(BF16)
            side_pos[(w, side)] = j + 1
        per_core.append(dict(lo_idx=lo_idx, hi_idx=hi_idx, s_idx=s_idx, s_val=s_val))

    # -------- vp / vc scattered-A streams --------
    def build_A(rows_a, cols_a, vals_a, nchunk):
        rows_a = np.asarray(rows_a, np.int64)
        cols_a = np.asarray(cols_a, np.int64)
        vals_a = np.asarray(vals_a, f32)
        # coalesce duplicate (row, col) entries (sparse-matrix canonicalization;
        # local_scatter cannot express two writes to one A cell)
        ncols = int(cols_a.max()) + 1 if len(cols_a) else 1
        key0 = rows_a * ncols + cols_a
        uk, inv = np.unique(key0, return_inverse=True)
        vsum = np.zeros(len(uk), f32)
        np.add.at(vsum, inv, vals_a)
        rows_a, cols_a, vals_a = uk // ncols, uk % ncols, vsum
        core_a = rows_a // c.NSH
        pos_a = rows_a % c.NSH
        t_a = pos_a // 512
        rel_a = pos_a % 512
        q_a = cols_a // P
        p_a = cols_a % P
        # K = max entries per (core,tile,chunk,partition)
        key = (((core_a * c.NTN + t_a) * nchunk + q_a) * P + p_a)
        cnts = np.bincount(key, minlength=c.NCORE * c.NTN * nchunk * P)
        K = int(cnts.max())
        K = max(2, K + (K % 2))
        idx_all, val_all = [], []
        for co in range(c.NCORE):
            idx = np.full((P, c.NTN * nchunk * K), -1, np.int16)
            val = np.zeros((P, c.NTN * nchunk * K), BF16)
            sel = core_a == co
            tt, qq, pp, rr, vv2 = t_a[sel], q_a[sel], p_a[sel], rel_a[sel], vals_a[sel]
            o2 = np.lexsort((rr, pp, qq, tt))
            tt, qq, pp, rr, vv2 = tt[o2], qq[o2], pp[o2], rr[o2], vv2[o2]
            slot = np.zeros(len(tt), np.int64)
            if len(tt):
                keyc = ((tt * nchunk + qq) * P + pp)
                # running index within each key group (sorted -> groups contiguous)
                change = np.ones(len(keyc), bool)
                change[1:] = keyc[1:] != keyc[:-1]
                gstart = np.flatnonzero(change)
                gid = np.cumsum(change) - 1
                slot = np.arange(len(keyc)) - gstart[gid]
                colpos = (tt * nchunk + qq) * K + slot
                idx[pp, colpos] = rr.astype(np.int16)
                val[pp, colpos] = vv2.astype(BF16)
            idx_all.append(idx)
            val_all.append(val)
        return K, idx_all, val_all

    vp_cols_p = inv_p[np.asarray(inputs['vp_cols'], np.int64)]
    vc_cols_c = inv_c[np.asarray(inputs['vc_cols'], np.int64)]
    KVP, vp_idx, vp_val = build_A(inputs['vp_rows'], vp_cols_p, inputs['vp_vals'], 1)
    KVC, vc_idx, vc_val = build_A(inputs['vc_rows'], vc_cols_c, inputs['vc_vals'], c.NQC)

    # -------- masks (transposed, permuted, padded, bf16) --------
    pvT = np.asarray(inputs['pv_dense'], f32).T[:, perm_p]       # (N, NP)
    cvT = np.asarray(inputs['cv_dense'], f32).T[:, perm_c]       # (N, NCAT)
    pcT = np.asarray(inputs['pc_dense'], f32).T[:, perm_p][perm_c, :]  # (NCAT, NP) rows cate
    cpT = np.asarray(inputs['cp_dense'], f32).T[:, perm_c][perm_p, :]  # (NP, NCAT) rows price
    maskL1 = []
    maskL2 = []
    for co in range(c.NCORE):
        sl = slice(co * c.NSH, (co + 1) * c.NSH)
        big = np.zeros((c.NSHP, c.NP + c.NCAT), BF16)
        big[:c.NSH, :c.NP] = _bf(pvT[sl])
        big[:c.NSH, c.NP:] = _bf(cvT[sl])
        maskL1.append(big)
        maskL2.append(np.ascontiguousarray(big[:, :c.NP]))
    pcT_m = np.zeros((c.NCATP, c.NP), BF16); pcT_m[:c.NCAT] = _bf(pcT)
    cpT_m = np.zeros((P, c.NCAT), BF16); cpT_m[:c.NP] = _bf(cpT)

    # -------- tables / weights / broadcast vectors --------
    W = {k: np.asarray(inputs[k], f32) for k in
         ['Wai', 'bai', 'Wi1', 'bi1', 'Wi2', 'bi2', 'Wap', 'bap', 'Wp1', 'bp1',
          'Wp2', 'bp2', 'Wac', 'bac', 'Wc1', 'bc1', 'Wc2', 'bc2']}
    def chunks(Wa):
        WaT = Wa.T  # (3*EMB, EMB)
        return [WaT[k * P:(k + 1) * P, :] for k in range(3)]
    Wstack = np.stack(chunks(W['Wai']) + [W['Wi1'].T, W['Wi2'].T]
                      + chunks(W['Wap']) + [W['Wp1'].T, W['Wp2'].T]
                      + chunks(W['Wac']) + [W['Wc1'].T, W['Wc2'].T]).astype(BF16)
    Bstack = np.stack([
        np.stack([W['bai'], W['bi1'], W['bi2']], axis=1),
        np.stack([W['bap'], W['bp1'], W['bp2']], axis=1),
        np.stack([W['bac'], W['bc1'], W['bc2']], axis=1)]).astype(f32)  # (3,128,3)

    pri_tab = np.zeros((P, EMB), BF16); pri_tab[:c.NP] = _bf(pri_s)
    cate_tab = np.zeros((c.NCATP, EMB), BF16); cate_tab[:c.NCAT] = _bf(cate_s)

    mstack = np.zeros((P, 2 * (c.NP + c.NCAT)), f32)
    o = 0
    for v in (m_pv_s, m_cv_s, m_pc, m_cp):
        mstack[:, o:o + len(v)] = v[None, :]
        o += len(v)

    rep = dict(
        emb_tab=emb_tab, pri_tab=pri_tab, cate_tab=cate_tab,
        priT32=np.ascontiguousarray(pri_s.T), priT16=np.ascontiguousarray(_bf(pri_s.T)),
        cateT32=np.ascontiguousarray(cate_s.T), cateT16=np.ascontiguousarray(_bf(cate_s.T)),
        Wstack=Wstack, Bstack=Bstack, mstack=mstack,
        pcT_m=pcT_m, cpT_m=cpT_m,
        ident=np.eye(P, dtype=BF16), ones_c=np.ones((P, 1), BF16),
        ones_r=np.ones((1, P), f32),
    )

    in_maps = []
    for co in range(c.NCORE):
        sl = slice(co * c.NSH, (co + 1) * c.NSH)
        embT32 = np.zeros((P, c.NPAD), f32)
        embT32[:, :c.NSH] = emb[sl].T
        emb_norm = np.zeros((c.NSHP, EMB), BF16)
        emb_norm[:c.NSH] = _bf(emb[sl])
        m = dict(rep)
        m.update(
            embT32=embT32, embT16=embT32.astype(BF16),
            emb_norm=emb_norm,
            maskL1=maskL1[co], maskL2=maskL2[co],
            glo_idx=_wrap_idx(per_core[co]['lo_idx']),
            ghi_idx=_wrap_idx(per_core[co]['hi_idx']),
            s_idx=per_core[co]['s_idx'], s_val=per_core[co]['s_val'],
            vp_idx=vp_idx[co], vp_val=vp_val[co],
            vc_idx=vc_idx[co], vc_val=vc_val[co],
        )
        in_maps.append(m)

    meta = dict(nlo=nlo.tolist(), nhi=nhi.tolist(), blocks_p=blocks_p,
                scalls=scalls, glo_calls=glo_calls, ghi_calls=ghi_calls,
                locnt=locnt, hicnt=hicnt, nblkp=nblkp,
                KVP=KVP, KVC=KVC, npos_p=npos_p, npos_c=npos_c,
                perm_p=perm_p, perm_c=perm_c,
                layers=int(np.asarray(inputs['layers'])))
    return in_maps, meta


# ---------------------------------------------------------------------------

def build(cfg: Cfg, meta):
    import concourse.bacc as bacc
    import concourse.bass as bass
    import concourse.tile as tile
    import concourse.mybir as mybir
    import concourse.bass_isa as bass_isa

    c = cfg
    dt = mybir.dt
    ALU = mybir.AluOpType
    ACTF = mybir.ActivationFunctionType
    AX = mybir.AxisListType
    layers = meta['layers']
    RG = [list(range(c.NCORE))]
    NP_, NCAT, NQC = c.NP, c.NCAT, c.NQC
    RC = NP_ + NCAT

    nc = bacc.Bacc("TRN2", target_bir_lowering=False, debug=False,
                   num_devices=c.NCORE)

    def din(name, shape, dtype):
        return nc.dram_tensor(name, list(shape), dtype, kind="ExternalInput")

    # ---- inputs ----
    emb_tab = din('emb_tab', (c.NTABP, EMB), dt.bfloat16)
    pri_tab_i = din('pri_tab', (P, EMB), dt.bfloat16)
    cate_tab_i = din('cate_tab', (c.NCATP, EMB), dt.bfloat16)
    priT32_i = din('priT32', (EMB, NP_), dt.float32)
    priT16_i = din('priT16', (EMB, NP_), dt.bfloat16)
    cateT32_i = din('cateT32', (EMB, NCAT), dt.float32)
    cateT16_i = din('cateT16', (EMB, NCAT), dt.bfloat16)
    Wstack = din('Wstack', (15, P, EMB), dt.bfloat16)
    Bstack = din('Bstack', (3, P, 3), dt.float32)
    mstack = din('mstack', (P, 2 * RC), dt.float32)
    pcT_m = din('pcT_m', (c.NCATP, NP_), dt.bfloat16)
    cpT_m = din('cpT_m', (P, NCAT), dt.bfloat16)
    ident_i = din('ident', (P, P), dt.bfloat16)
    ones_c_i = din('ones_c', (P, 1), dt.bfloat16)
    ones_r_i = din('ones_r', (1, P), dt.float32)
    embT32_i = din('embT32', (P, c.NPAD), dt.float32)
    embT16_i = din('embT16', (P, c.NPAD), dt.bfloat16)
    emb_norm_i = din('emb_norm', (c.NSHP, EMB), dt.bfloat16)
    maskL1_i = din('maskL1', (c.NSHP, RC), dt.bfloat16)
    maskL2_i = din('maskL2', (c.NSHP, NP_), dt.bfloat16)
    glo_i = din('glo_idx', (P, max(16, meta['locnt'] * P // 16)), dt.int16)
    ghi_i = din('ghi_idx', (P, max(16, meta['hicnt'] * P // 16)), dt.int16)
    s_idx_i = din('s_idx', (P, meta['nblkp']), dt.int16)
    s_val_i = din('s_val', (P, meta['nblkp']), dt.bfloat16)
    vp_idx_i = din('vp_idx', (P, c.NTN * meta['KVP']), dt.int16)
    vp_val_i = din('vp_val', (P, c.NTN * meta['KVP']), dt.bfloat16)
    vc_idx_i = din('vc_idx', (P, c.NTN * NQC * meta['KVC']), dt.int16)
    vc_val_i = din('vc_val', (P, c.NTN * NQC * meta['KVC']), dt.bfloat16)

    item_t = nc.dram_tensor('item_t', [P, c.NSH], dt.float32, kind="ExternalOutput")
    price_t = nc.dram_tensor('price_t', [P, NP_], dt.float32, kind="ExternalOutput")

    # per-layer collective tensors
    ccmax_in = [nc.dram_tensor(f'ccmax_in{L}', [1, 8], dt.float32) for L in range(layers)]
    ccmax_out = [nc.dram_tensor(f'ccmax_out{L}', [1, 8], dt.float32, addr_space="Shared")
                 for L in range(layers)]
    arp_in, arp_out = [], []
    for L in range(layers):
        rc = RC if L < layers - 1 else NP_
        arp_in.append(nc.dram_tensor(f'arp_in{L}', [P + 1, rc], dt.float32))
        arp_out.append(nc.dram_tensor(f'arp_out{L}', [P + 1, rc], dt.float32,
                                      addr_space="Shared"))
    ag_in = [nc.dram_tensor(f'ag_in{L}', [c.NSHP, EMB], dt.bfloat16)
             for L in range(max(0, layers - 1))]
    ag_out = [nc.dram_tensor(f'ag_out{L}', [c.NTABP, EMB], dt.bfloat16,
                             addr_space="Shared")
              for L in range(max(0, layers - 1))]
    itemT32_d = [nc.dram_tensor(f'itemT32_{L}', [P, c.NPAD], dt.float32)
                 for L in range(max(0, layers - 1))]
    itemT16_d = [nc.dram_tensor(f'itemT16_{L}', [P, c.NPAD], dt.bfloat16)
                 for L in range(max(0, layers - 1))]

    nlo, nhi = meta['nlo'], meta['nhi']
    blocks_p, scalls = meta['blocks_p'], meta['scalls']
    glo_calls, ghi_calls = meta['glo_calls'], meta['ghi_calls']

    from contextlib import ExitStack
    with tile.TileContext(nc) as tc, ExitStack() as es:
        cp_ = es.enter_context(tc.tile_pool(name="const", bufs=1))
        sp = es.enter_context(tc.tile_pool(name="stream", bufs=1))
        pp = es.enter_context(tc.tile_pool(name="ps", bufs=1, space="PSUM"))

        def dma(out_ap, in_ap):
            nc.sync.dma_start(out_ap, in_ap)

        # ---------------- constants ----------------
        W_sb = cp_.tile([P, 15, EMB], dt.bfloat16)
        dma(W_sb[:], Wstack[:, :, :].rearrange("i p q -> p i q"))
        bias3 = cp_.tile([P, 3, 3], dt.float32)
        dma(bias3[:], Bstack[:, :, :].rearrange("i p q -> p i q"))
        bcomb = cp_.tile([P, 3], dt.float32)
        for br in range(3):
            nc.vector.tensor_reduce(bcomb[:, br:br + 1], bias3[:, br, :],
                                    axis=AX.X, op=ALU.add)
        ident = cp_.tile([P, P], dt.bfloat16); dma(ident[:], ident_i[:, :])
        ones_c = cp_.tile([P, 1], dt.bfloat16); dma(ones_c[:], ones_c_i[:, :])
        ones_r = cp_.tile([1, P], dt.float32); dma(ones_r[:], ones_r_i[:, :])
        m_sb = cp_.tile([P, 2 * RC], dt.float32); dma(m_sb[:], mstack[:, :])
        m_pv_b = m_sb[:, 0:NP_]
        m_cv_b = m_sb[:, NP_:RC]
        m_pc_b = m_sb[:, RC:RC + NP_]
        m_cp_b = m_sb[:, RC + NP_:2 * RC]
        pcT_sb = cp_.tile([P, NQC, NP_], dt.bfloat16)
        dma(pcT_sb[:], pcT_m[:, :].rearrange("(q p) r -> p q r", p=P))
        cpT_sb = cp_.tile([P, NCAT], dt.bfloat16); dma(cpT_sb[:], cpT_m[:, :])
        pri_norm0 = cp_.tile([P, EMB], dt.bfloat16); dma(pri_norm0[:], pri_tab_i[:, :])
        cate_norm0 = cp_.tile([P, NQC, EMB], dt.bfloat16)
        dma(cate_norm0[:], cate_tab_i[:, :].rearrange("(q p) e -> p q e", p=P))
        priT32_0 = cp_.tile([P, NP_], dt.float32); dma(priT32_0[:], priT32_i[:, :])
        priT16_0 = cp_.tile([P, NP_], dt.bfloat16); dma(priT16_0[:], priT16_i[:, :])
        cateT32_0 = cp_.tile([P, NCAT], dt.float32); dma(cateT32_0[:], cateT32_i[:, :])
        cateT16_0 = cp_.tile([P, NCAT], dt.bfloat16); dma(cateT16_0[:], cateT16_i[:, :])
        glo_sb = cp_.tile([P, glo_i.shape[1]], dt.int16); dma(glo_sb[:], glo_i[:, :])
        ghi_sb = cp_.tile([P, ghi_i.shape[1]], dt.int16); dma(ghi_sb[:], ghi_i[:, :])
        s_idx_sb = cp_.tile([P, meta['nblkp']], dt.int16); dma(s_idx_sb[:], s_idx_i[:, :])
        s_val_sb = cp_.tile([P, meta['nblkp']], dt.bfloat16); dma(s_val_sb[:], s_val_i[:, :])
        vp_idx_sb = cp_.tile([P, c.NTN * meta['KVP']], dt.int16); dma(vp_idx_sb[:], vp_idx_i[:, :])
        vp_val_sb = cp_.tile([P, c.NTN * meta['KVP']], dt.bfloat16); dma(vp_val_sb[:], vp_val_i[:, :])
        vc_idx_sb = cp_.tile([P, c.NTN * NQC * meta['KVC']], dt.int16); dma(vc_idx_sb[:], vc_idx_i[:, :])
        vc_val_sb = cp_.tile([P, c.NTN * NQC * meta['KVC']], dt.bfloat16); dma(vc_val_sb[:], vc_val_i[:, :])

        # e1 / norm-tile sources stay in DRAM and are streamed per tile
        state = dict(e1d32=embT32_i, e1d16=embT16_i, normd=emb_norm_i,
                     priT32=priT32_0, priT16=priT16_0, pri_norm=pri_norm0,
                     cateT32=cateT32_0, cateT16=cateT16_0, cate_norm=cate_norm0,
                     tab=emb_tab)

        for L in range(layers):
            last = (L == layers - 1)
            rc = RC if not last else NP_
            mask_i = maskL1_i if not last else maskL2_i

            # ================= phase S: stats =================
            s_col = sp.tile([P, c.JT], dt.float32, tag="s_col", bufs=2)
            for j in range(c.JT):
                nrm_s = sp.tile([P, EMB], dt.bfloat16, tag="nrm_s", bufs=3)
                dma(nrm_s[:], state['normd'][j * P:(j + 1) * P, :])
                nc.vector.tensor_reduce(s_col[:, j:j + 1], nrm_s[:],
                                        axis=AX.X, op=ALU.add)
            smax_l = sp.tile([P, 1], dt.float32, tag="st1", bufs=4)
            nc.vector.tensor_reduce(smax_l[:], s_col[:], axis=AX.X, op=ALU.max)
            nc.gpsimd.partition_all_reduce(smax_l[:], smax_l[:], 128,
                                           bass_isa.ReduceOp.max)
            smin_l = sp.tile([P, 1], dt.float32, tag="st1", bufs=4)
            nc.vector.tensor_reduce(smin_l[:], s_col[:], axis=AX.X, op=ALU.min)
            nc.vector.tensor_scalar_mul(smin_l[:], smin_l[:], -1.0)
            nc.gpsimd.partition_all_reduce(smin_l[:], smin_l[:], 128,
                                           bass_isa.ReduceOp.max)
            stage = sp.tile([1, 8], dt.float32, tag="st8", bufs=2)
            nc.vector.memset(stage[:], 0.0)
            nc.vector.tensor_copy(stage[0:1, 0:1], smax_l[0:1, :])
            nc.vector.tensor_copy(stage[0:1, 1:2], smin_l[0:1, :])
            dma(ccmax_in[L][:, :], stage[:])
            nc.gpsimd.collective_compute(
                "AllReduce", ALU.max, replica_groups=RG,
                ins=[ccmax_in[L][:, :].opt()], outs=[ccmax_out[L][:, :].opt()])
            strow = sp.tile([1, 8], dt.float32, tag="st8", bufs=2)
            dma(strow[:], ccmax_out[L][:, :])
            stbc = sp.tile([P, 8], dt.float32, tag="stbc", bufs=2)
            nc.gpsimd.partition_broadcast(stbc[:], strow[:])
            smax_bc = stbc[:, 0:1]
            smin_bc = sp.tile([P, 1], dt.float32, tag="st1", bufs=4)
            nc.vector.tensor_scalar_mul(smin_bc[:], stbc[:, 1:2], -1.0)
            SSmax = sp.tile([P, c.JT], dt.float32, tag="ssm", bufs=2)
            nc.vector.tensor_scalar(SSmax[:], s_col[:], smax_bc, None, op0=ALU.subtract)
            SSmin = sp.tile([P, c.JT], dt.float32, tag="ssn", bufs=2)
            nc.vector.tensor_scalar(SSmin[:], s_col[:], smin_bc[:], None, op0=ALU.subtract)

            # local stats for pc (cate table) and cp (price table)
            s_cate = sp.tile([P, NQC], dt.float32, tag="s_cate", bufs=2)
            for q in range(NQC):
                nc.vector.tensor_reduce(s_cate[:, q:q + 1], state['cate_norm'][:, q, :],
                                        axis=AX.X, op=ALU.add)
            cmax = sp.tile([P, 1], dt.float32, tag="st1", bufs=4)
            nc.vector.tensor_reduce(cmax[:], s_cate[:], axis=AX.X, op=ALU.max)
            nc.gpsimd.partition_all_reduce(cmax[:], cmax[:], 128, bass_isa.ReduceOp.max)
            cmin = sp.tile([P, 1], dt.float32, tag="st1", bufs=4)
            nc.vector.tensor_reduce(cmin[:], s_cate[:], axis=AX.X, op=ALU.min)
            nc.vector.tensor_scalar_mul(cmin[:], cmin[:], -1.0)
            nc.gpsimd.partition_all_reduce(cmin[:], cmin[:], 128, bass_isa.ReduceOp.max)
            nc.vector.tensor_scalar_mul(cmin[:], cmin[:], -1.0)
            c_pc = sp.tile([P, NP_], dt.float32, tag="c_pc", bufs=2)
            t1 = sp.tile([P, NCAT], dt.float32, tag="ctmp", bufs=2)
            nc.vector.tensor_scalar(c_pc[:], m_pc_b, cmax[:], None, op0=ALU.mult)
            nc.vector.tensor_scalar(t1[:, :NP_], m_pc_b, cmin[:], None, op0=ALU.mult)
            nc.vector.tensor_max(c_pc[:], c_pc[:], t1[:, :NP_])
            c_cp = None
            if not last:
                s_pri = sp.tile([P, 1], dt.float32, tag="st1", bufs=4)
                nc.vector.tensor_reduce(s_pri[:], state['pri_norm'][:], axis=AX.X, op=ALU.add)
                pmax = sp.tile([P, 1], dt.float32, tag="st1", bufs=4)
                nc.vector.tensor_copy(pmax[:], s_pri[:])
                nc.gpsimd.partition_all_reduce(pmax[:], pmax[:], 128, bass_isa.ReduceOp.max)
                pmin = sp.tile([P, 1], dt.float32, tag="st1", bufs=4)
                nc.vector.tensor_scalar_mul(pmin[:], s_pri[:], -1.0)
                nc.gpsimd.partition_all_reduce(pmin[:], pmin[:], 128, bass_isa.ReduceOp.max)
                nc.vector.tensor_scalar_mul(pmin[:], pmin[:], -1.0)
                c_cp = sp.tile([P, NCAT], dt.float32, tag="c_cp", bufs=2)
                nc.vector.tensor_scalar(c_cp[:], m_cp_b, pmax[:], None, op0=ALU.mult)
                nc.vector.tensor_scalar(t1[:], m_cp_b, pmin[:], None, op0=ALU.mult)
                nc.vector.tensor_max(c_cp[:], c_cp[:], t1[:])
                # s_pri per-partition scalars for cp intra (j = price rows)
                state['s_pri'] = s_pri
            state['s_cate'] = s_cate

            # ================= phase A: spmm + item gate =================
            gcall_done = [0, 0]   # lo, hi calls issued
            g_tiles = [{}, {}]
            scall_done = 0
            s_tiles = {}

            def issue_gather(side, callidx):
                calls = glo_calls if side == 0 else ghi_calls
                s0, nb = calls[callidx]
                gt = sp.tile([P, c.GCH, EMB], dt.bfloat16,
                             tag=f"g{side}", bufs=2)
                idxs = (glo_sb if side == 0 else ghi_sb)
                src = state['tab'][:, :] if side == 0 else state['tab'][c.LOH:, :]
                nc.gpsimd.dma_gather(
                    out_ap=gt[:, 0:nb, :],
                    in_ap=src,
                    idxs_ap=idxs[:, s0 * 8: s0 * 8 + nb * 8],
                    num_idxs=nb * P, num_idxs_reg=nb * P,
                    elem_size=EMB)
                g_tiles[side][callidx] = gt

            def issue_scatter(callidx):
                k0, nb = scalls[callidx]
                st = sp.tile([P, c.SG * P], dt.bfloat16, tag="sblk", bufs=3)
                nc.gpsimd.local_scatter(
                    out_ap=st[:, 0:nb * P],
                    data_ap=s_val_sb[:, k0:k0 + nb],
                    idxs_ap=s_idx_sb[:, k0:k0 + nb],
                    channels=P, num_elems=nb * P, num_idxs=nb)
                s_tiles[callidx] = st

            kptr = 0   # walks blocks_p
            for t in range(c.NTN):
                # ---- e2 (vp) / e3 (vc) ----
                KVP, KVC = meta['KVP'], meta['KVC']
                a_vp = sp.tile([P, 512], dt.bfloat16, tag="avp", bufs=2)
                nc.gpsimd.local_scatter(
                    out_ap=a_vp[:], data_ap=vp_val_sb[:, t * KVP:(t + 1) * KVP],
                    idxs_ap=vp_idx_sb[:, t * KVP:(t + 1) * KVP],
                    channels=P, num_elems=512, num_idxs=KVP)
                e2_ps = pp.tile([P, 512], dt.float32, tag="e2ps")
                nc.tensor.matmul(e2_ps[:], state['pri_norm'][:], a_vp[:],
                                 start=True, stop=True)
                e3_ps = pp.tile([P, 512], dt.float32, tag="e3ps")
                for q in range(NQC):
                    a_vc = sp.tile([P, 512], dt.bfloat16, tag="avc", bufs=2)
                    o3 = (t * NQC + q) * KVC
                    nc.gpsimd.local_scatter(
                        out_ap=a_vc[:], data_ap=vc_val_sb[:, o3:o3 + KVC],
                        idxs_ap=vc_idx_sb[:, o3:o3 + KVC],
                        channels=P, num_elems=512, num_idxs=KVC)
                    nc.tensor.matmul(e3_ps[:], state['cate_norm'][:, q, :], a_vc[:],
                                     start=(q == 0), stop=(q == NQC - 1))
                e2_16 = sp.tile([P, 512], dt.bfloat16, tag="e2_16", bufs=2)
                nc.scalar.copy(e2_16[:], e2_ps[:])
                e3_16 = sp.tile([P, 512], dt.bfloat16, tag="e3_16", bufs=2)
                nc.scalar.copy(e3_16[:], e3_ps[:])

                # ---- adj windows ----
                adj_ps = pp.tile([P, 512], dt.float32, tag="adjps")
                for wl in range(4):
                    w = t * 4 + wl
                    wblocks = [(kp, blk) for kp, blk in enumerate(blocks_p)
                               if blk[0] == w]
                    if not wblocks:
                        nc.vector.memset(adj_ps[:, wl * P:(wl + 1) * P], 0.0)
                        continue
                    nb_w = len(wblocks)
                    for bi, (kp, (bw, side, slot)) in enumerate(wblocks):
                        gcall = slot // c.GCH
                        brel = slot % c.GCH
                        while gcall_done[side] <= gcall:
                            issue_gather(side, gcall_done[side])
                            gcall_done[side] += 1
                        sc = next(i for i, (k0, nb) in enumerate(scalls)
                                  if k0 <= kp < k0 + nb)
                        while scall_done <= sc:
                            issue_scatter(scall_done)
                            scall_done += 1
                        k0 = scalls[sc][0]
                        gt = g_tiles[side][gcall]
                        st = s_tiles[sc]
                        nc.tensor.matmul(
                            adj_ps[:, wl * P:(wl + 1) * P],
                            gt[:, brel, :],
                            st[:, (kp - k0) * P:(kp - k0 + 1) * P],
                            start=(bi == 0), stop=(bi == nb_w - 1))

                # ---- gate ----
                gate_ps = pp.tile([P, 512], dt.float32, tag="gatetr")
                nsl = slice(t * 512, (t + 1) * 512)
                e1t16 = sp.tile([P, 512], dt.bfloat16, tag="e1t16", bufs=2)
                dma(e1t16[:], state['e1d16'][:, nsl])
                rhs_list = [e1t16[:], e2_16[:], e3_16[:], e2_16[:], e3_16[:]]
                for i5 in range(5):
                    nc.tensor.matmul(gate_ps[:], W_sb[:, i5, :], rhs_list[i5],
                                     start=(i5 == 0), stop=(i5 == 4))
                g_sb = sp.tile([P, 512], dt.float32, tag="g_sb", bufs=2)
                nc.scalar.activation(g_sb[:], gate_ps[:], ACTF.Sigmoid,
                                     bias=bcomb[:, 0:1])
                # ---- combine: item = e1 + e3 + g*(e2-e3) + adj ----
                e1t32 = sp.tile([P, 512], dt.float32, tag="e1t32", bufs=2)
                dma(e1t32[:], state['e1d32'][:, nsl])
                e2_32 = sp.tile([P, 512], dt.float32, tag="e2_32", bufs=2)
                nc.scalar.copy(e2_32[:], e2_ps[:])
                x = sp.tile([P, 512], dt.float32, tag="xcmb", bufs=2)
                nc.vector.tensor_sub(x[:], e2_32[:], e3_ps[:])
                nc.vector.tensor_mul(x[:], x[:], g_sb[:])
                nc.vector.tensor_add(x[:], x[:], e1t32[:])
                nc.vector.tensor_add(x[:], x[:], e3_ps[:])
                xo = sp.tile([P, 512], dt.float32, tag="xout", bufs=2)
                nc.vector.tensor_add(xo[:], x[:], adj_ps[:])
                if not last:
                    dma(itemT32_d[L][:, nsl], xo[:])
                    it16 = sp.tile([P, 512], dt.bfloat16, tag="it16", bufs=2)
                    nc.scalar.copy(it16[:], xo[:])
                    dma(itemT16_d[L][:, nsl], it16[:])
                    normt = sp.tile([P, 4, EMB], dt.bfloat16, tag="normt", bufs=2)
                    for q4 in range(4):
                        j = t * 4 + q4
                        if j >= c.JT:
                            continue
                        tr_ps = pp.tile([P, P], dt.bfloat16, tag="gatetr")
                        nc.tensor.transpose(tr_ps[:],
                                            it16[:, q4 * P:(q4 + 1) * P], ident[:])
                        nc.vector.tensor_copy(normt[:, q4, :], tr_ps[:])
                        dma(ag_in[L][j * P:(j + 1) * P, :], normt[:, q4, :])
                else:
                    lo_n = t * 512
                    hi_n = min((t + 1) * 512, c.NSH)
                    if hi_n > lo_n:
                        dma(item_t[:, lo_n:hi_n], xo[:, 0:hi_n - lo_n])

            if not last:
                nc.gpsimd.collective_compute(
                    "AllGather", ALU.bypass, replica_groups=RG,
                    ins=[ag_in[L][:, :].opt()], outs=[ag_out[L][:, :].opt()])

            # ================= phase B: pv (+cv) contraction =================
            pv_ps = pp.tile([P, NP_], dt.float32, tag="acc1")
            dpv_ps = pp.tile([1, NP_], dt.float32, tag="d1")
            cv_ps = dcv_ps = None
            if not last:
                cv_ps = pp.tile([P, NCAT], dt.float32, tag="acc2")
                dcv_ps = pp.tile([1, NCAT], dt.float32, tag="d2")
            npos_p, npos_c = meta['npos_p'], meta['npos_c']
            for j in range(c.JT):
                mk = sp.tile([P, rc], dt.bfloat16, tag="mask", bufs=3)
                dma(mk[:], mask_i[j * P:(j + 1) * P, :])
                nrm_b = sp.tile([P, EMB], dt.bfloat16, tag="nrm_b", bufs=3)
                dma(nrm_b[:], state['normd'][j * P:(j + 1) * P, :])
                tf = sp.tile([P, rc], dt.float32, tag="tf", bufs=2)
                if npos_p:
                    nc.vector.tensor_scalar(tf[:, 0:npos_p], m_pv_b[:, 0:npos_p],
                                            SSmax[:, j:j + 1], None, op0=ALU.mult)
                if npos_p < NP_:
                    nc.vector.tensor_scalar(tf[:, npos_p:NP_], m_pv_b[:, npos_p:NP_],
                                            SSmin[:, j:j + 1], None, op0=ALU.mult)
                if not last:
                    if npos_c:
                        nc.vector.tensor_scalar(tf[:, NP_:NP_ + npos_c],
                                                m_cv_b[:, 0:npos_c],
                                                SSmax[:, j:j + 1], None, op0=ALU.mult)
                    if npos_c < NCAT:
                        nc.vector.tensor_scalar(tf[:, NP_ + npos_c:],
                                                m_cv_b[:, npos_c:],
                                                SSmin[:, j:j + 1], None, op0=ALU.mult)
                eb = sp.tile([P, rc], dt.bfloat16, tag="eb", bufs=2)
                nc.scalar.activation(eb[:], tf[:], ACTF.Exp)
                em = sp.tile([P, rc], dt.bfloat16, tag="em", bufs=2)
                nc.vector.tensor_mul(em[:], eb[:], mk[:])
                st_, sp_ = (j == 0), (j == c.JT - 1)
                nc.tensor.matmul(pv_ps[:], nrm_b[:], em[:, 0:NP_],
                                 start=st_, stop=sp_)
                nc.tensor.matmul(dpv_ps[:], ones_c[:], em[:, 0:NP_],
                                 start=st_, stop=sp_)
                if not last:
                    nc.tensor.matmul(cv_ps[:], nrm_b[:], em[:, NP_:],
                                     start=st_, stop=sp_)
                    nc.tensor.matmul(dcv_ps[:], ones_c[:], em[:, NP_:],
                                     start=st_, stop=sp_)
            nums = sp.tile([P, rc], dt.float32, tag="nums", bufs=2)
            nc.scalar.copy(nums[:, 0:NP_], pv_ps[:])
            dens = sp.tile([1, rc], dt.float32, tag="dens", bufs=2)
            nc.vector.tensor_copy(dens[0:1, 0:NP_], dpv_ps[:])
            if not last:
                nc.scalar.copy(nums[:, NP_:], cv_ps[:])
                nc.vector.tensor_copy(dens[0:1, NP_:], dcv_ps[:])
            dma(arp_in[L][0:P, :], nums[:])
            dma(arp_in[L][P:P + 1, :], dens[:])
            nc.gpsimd.collective_compute(
                "AllReduce", ALU.add, replica_groups=RG,
                ins=[arp_in[L][:, :].opt()], outs=[arp_out[L][:, :].opt()])
            numsR = sp.tile([P, rc], dt.float32, tag="numsR", bufs=2)
            dma(numsR[:], arp_out[L][0:P, :])
            densR = sp.tile([1, rc], dt.float32, tag="densR", bufs=2)
            dma(densR[:], arp_out[L][P:P + 1, :])
            recip = sp.tile([1, rc], dt.float32, tag="recip", bufs=2)
            nc.vector.reciprocal(recip[:], densR[:])
            e2pT32 = sp.tile([P, NP_], dt.float32, tag="e2pT32", bufs=2)
            e2pT16 = sp.tile([P, NP_], dt.bfloat16, tag="e2pT16", bufs=2)
            bc_ps = pp.tile([P, NP_], dt.float32, tag="d1")
            nc.tensor.matmul(bc_ps[:], ones_r[:], recip[0:1, 0:NP_], start=True, stop=True)
            nc.vector.tensor_mul(e2pT32[:], numsR[:, 0:NP_], bc_ps[:])
            nc.scalar.copy(e2pT16[:], e2pT32[:])
            if not last:
                e3cT32 = sp.tile([P, NCAT], dt.float32, tag="e3cT32", bufs=2)
                e3cT16 = sp.tile([P, NCAT], dt.bfloat16, tag="e3cT16", bufs=2)
                bc2_ps = pp.tile([P, NCAT], dt.float32, tag="d2")
                nc.tensor.matmul(bc2_ps[:], ones_r[:], recip[0:1, NP_:], start=True, stop=True)
                nc.vector.tensor_mul(e3cT32[:], numsR[:, NP_:], bc2_ps[:])
                nc.scalar.copy(e3cT16[:], e3cT32[:])

            # ================= phase C: pc/cp intra + price/cate gates =======
            # pc intra (j = cate rows): e3 of price branch
            pcn_ps = pp.tile([P, NP_], dt.float32, tag="acc1")
            pcdS_ps = pp.tile([1, NP_], dt.float32, tag="d1")
            pcdZ_ps = pp.tile([1, NP_], dt.float32, tag="d2")
            for q in range(NQC):
                tf2 = sp.tile([P, NP_], dt.float32, tag="tf2", bufs=2)
                nc.vector.tensor_scalar(tf2[:], m_pc_b, state['s_cate'][:, q:q + 1],
                                        None, op0=ALU.mult)
                nc.vector.tensor_sub(tf2[:], tf2[:], c_pc[:])
                eb2 = sp.tile([P, NP_], dt.bfloat16, tag="eb2", bufs=2)
                nc.scalar.activation(eb2[:], tf2[:], ACTF.Exp)
                em2 = sp.tile([P, NP_], dt.bfloat16, tag="em2", bufs=2)
                nc.vector.tensor_mul(em2[:], eb2[:], pcT_sb[:, q, :])
                st_, sp_ = (q == 0), (q == NQC - 1)
                nc.tensor.matmul(pcn_ps[:], state['cate_norm'][:, q, :], em2[:],
                                 start=st_, stop=sp_)
                nc.tensor.matmul(pcdS_ps[:], ones_c[:], em2[:], start=st_, stop=sp_)
                nc.tensor.matmul(pcdZ_ps[:], ones_c[:], eb2[:], start=st_, stop=sp_)
            dpc = sp.tile([1, NP_], dt.float32, tag="dpc", bufs=2)
            nc.vector.tensor_scalar(dpc[:], pcdZ_ps[:], 1e-8, None, op0=ALU.mult)
            nc.vector.tensor_add(dpc[:], dpc[:], pcdS_ps[:])
            nc.vector.reciprocal(dpc[:], dpc[:])
            bc3_ps = pp.tile([P, NP_], dt.float32, tag="d1")
            nc.tensor.matmul(bc3_ps[:], ones_r[:], dpc[:], start=True, stop=True)
            pcn_sb = sp.tile([P, NP_], dt.float32, tag="pcn_sb", bufs=2)
            nc.scalar.copy(pcn_sb[:], pcn_ps[:])
            e3pT32 = sp.tile([P, NP_], dt.float32, tag="e3pT32", bufs=2)
            nc.vector.tensor_mul(e3pT32[:], pcn_sb[:], bc3_ps[:])
            e3pT16 = sp.tile([P, NP_], dt.bfloat16, tag="e3pT16", bufs=2)
            nc.scalar.copy(e3pT16[:], e3pT32[:])

            # price gate
            pg_ps = pp.tile([P, NP_], dt.float32, tag="acc2")
            rhs5 = [state['priT16'][:], e2pT16[:], e3pT16[:], e2pT16[:], e3pT16[:]]
            for i5 in range(5):
                nc.tensor.matmul(pg_ps[:], W_sb[:, 5 + i5, :], rhs5[i5],
                                 start=(i5 == 0), stop=(i5 == 4))
            gp_sb = sp.tile([P, NP_], dt.float32, tag="gp_sb", bufs=2)
            nc.scalar.activation(gp_sb[:], pg_ps[:], ACTF.Sigmoid, bias=bcomb[:, 1:2])
            xp = sp.tile([P, NP_], dt.float32, tag="xp", bufs=2)
            nc.vector.tensor_sub(xp[:], e2pT32[:], e3pT32[:])
            nc.vector.tensor_mul(xp[:], xp[:], gp_sb[:])
            nc.vector.tensor_add(xp[:], xp[:], state['priT32'][:])
            nc.vector.tensor_add(xp[:], xp[:], e3pT32[:])
            if last:
                dma(price_t[:, :], xp[:])
            else:
                priT32n = cp_.tile([P, NP_], dt.float32, tag=f"priT32_{L % 2 + 1}")
                nc.vector.tensor_copy(priT32n[:], xp[:])
                priT16n = cp_.tile([P, NP_], dt.bfloat16, tag=f"priT16_{L % 2 + 1}")
                nc.scalar.copy(priT16n[:], xp[:])
                prn_ps = pp.tile([P, P], dt.bfloat16, tag="d1")
                nc.tensor.transpose(prn_ps[0:NP_, :], priT16n[:], ident[:])
                pri_normN = cp_.tile([P, EMB], dt.bfloat16, tag=f"pri_norm_{L % 2 + 1}")
                nc.vector.memset(pri_normN[:], 0.0)
                nc.vector.tensor_copy(pri_normN[0:NP_, :], prn_ps[0:NP_, :])

                # cp intra (j = price rows): e2 of cate branch
                tf3 = sp.tile([P, NCAT], dt.float32, tag="tf3", bufs=2)
                nc.vector.tensor_scalar(tf3[:], m_cp_b, state['s_pri'][:],
                                        None, op0=ALU.mult)
                nc.vector.tensor_sub(tf3[:], tf3[:], c_cp[:])
                eb3 = sp.tile([P, NCAT], dt.bfloat16, tag="eb3", bufs=2)
                nc.scalar.activation(eb3[:], tf3[:], ACTF.Exp)
                em3 = sp.tile([P, NCAT], dt.bfloat16, tag="em3", bufs=2)
                nc.vector.tensor_mul(em3[:], eb3[:], cpT_sb[:])
                cpn_ps = pp.tile([P, NCAT], dt.float32, tag="acc1")
                nc.tensor.matmul(cpn_ps[:], state['pri_norm'][:], em3[:],
                                 start=True, stop=True)
                cpdS_ps = pp.tile([1, NCAT], dt.float32, tag="d1")
                nc.tensor.matmul(cpdS_ps[:], ones_c[:], em3[:], start=True, stop=True)
                cpdZ_ps = pp.tile([1, NCAT], dt.float32, tag="d2")
                nc.tensor.matmul(cpdZ_ps[:], ones_c[:], eb3[:], start=True, stop=True)
                dcp = sp.tile([1, NCAT], dt.float32, tag="dcp", bufs=2)
                nc.vector.tensor_scalar(dcp[:], cpdZ_ps[:], 1e-8, None, op0=ALU.mult)
                nc.vector.tensor_add(dcp[:], dcp[:], cpdS_ps[:])
                nc.vector.reciprocal(dcp[:], dcp[:])
                bc4_ps = pp.tile([P, NCAT], dt.float32, tag="d1")
                nc.tensor.matmul(bc4_ps[:], ones_r[:], dcp[:], start=True, stop=True)
                cpn_sb = sp.tile([P, NCAT], dt.float32, tag="cpn_sb", bufs=2)
                nc.scalar.copy(cpn_sb[:], cpn_ps[:])
                e2cT32 = sp.tile([P, NCAT], dt.float32, tag="e2cT32", bufs=2)
                nc.vector.tensor_mul(e2cT32[:], cpn_sb[:], bc4_ps[:])
                e2cT16 = sp.tile([P, NCAT], dt.bfloat16, tag="e2cT16", bufs=2)
                nc.scalar.copy(e2cT16[:], e2cT32[:])

                # cate gate
                cg_ps = pp.tile([P, NCAT], dt.float32, tag="acc2")
                rhs5c = [state['cateT16'][:], e2cT16[:], e3cT16[:], e2cT16[:], e3cT16[:]]
                for i5 in range(5):
                    nc.tensor.matmul(cg_ps[:], W_sb[:, 10 + i5, :], rhs5c[i5],
                                     start=(i5 == 0), stop=(i5 == 4))
                gc_sb = sp.tile([P, NCAT], dt.float32, tag="gc_sb", bufs=2)
                nc.scalar.activation(gc_sb[:], cg_ps[:], ACTF.Sigmoid, bias=bcomb[:, 2:3])
                xc = sp.tile([P, NCAT], dt.float32, tag="xc", bufs=2)
                nc.vector.tensor_sub(xc[:], e2cT32[:], e3cT32[:])
                nc.vector.tensor_mul(xc[:], xc[:], gc_sb[:])
                nc.vector.tensor_add(xc[:], xc[:], state['cateT32'][:])
                nc.vector.tensor_add(xc[:], xc[:], e3cT32[:])
                cateT32n = cp_.tile([P, NCAT], dt.float32, tag=f"cateT32_{L % 2 + 1}")
                nc.vector.tensor_copy(cateT32n[:], xc[:])
                cateT16n = cp_.tile([P, NCAT], dt.bfloat16, tag=f"cateT16_{L % 2 + 1}")
                nc.scalar.copy(cateT16n[:], xc[:])
                cate_normN = cp_.tile([P, NQC, EMB], dt.bfloat16, tag=f"cate_norm_{L % 2 + 1}")
                for q in range(NQC):
                    lo_q = q * P
                    hi_q = min((q + 1) * P, NCAT)
                    n_q = hi_q - lo_q
                    cn_ps = pp.tile([P, P], dt.bfloat16, tag="d2")
                    nc.tensor.transpose(cn_ps[0:n_q, :], cateT16n[:, lo_q:hi_q], ident[:])
                    if n_q < P:
                        nc.vector.memset(cate_normN[:, q, :], 0.0)
                    nc.vector.tensor_copy(cate_normN[0:n_q, q, :], cn_ps[0:n_q, :])

                state = dict(e1d32=itemT32_d[L], e1d16=itemT16_d[L],
                             normd=ag_in[L],
                             priT32=priT32n, priT16=priT16n, pri_norm=pri_normN,
                             cateT32=cateT32n, cateT16=cateT16n, cate_norm=cate_normN,
                             tab=ag_out[L])

    nc.compile()
    return nc


# ---------------------------------------------------------------------------

_CACHE = {}


def run_on_hw(inputs, cfg=None, trace=False):
    from concourse import bass_utils
    cfg = cfg or Cfg()
    in_maps, meta = prep(inputs, cfg)
    nc = build(cfg, meta)
    res = bass_utils.run_bass_kernel_spmd(
        nc, in_maps, core_ids=list(range(cfg.NCORE)), trace=trace)
    return res, meta, cfg


def _assemble(results, meta, cfg):
    item = np.concatenate(
        [np.asarray(results[co]['item_t'], np.float32).T for co in range(cfg.NCORE)],
        axis=0)
    price_perm = np.asarray(results[0]['price_t'], np.float32)[:, :cfg.NP].T
    price = np.empty_like(price_perm)
    price[meta['perm_p']] = price_perm
    return item, price


def kernel(**inputs):
    res, meta, cfg = run_on_hw(inputs)
    return _assemble(res.results, meta, cfg)


# revision 24
# speedup vs baseline: 1.8382x; 1.8382x over previous
"""CoHHN GNN message-passing kernel for 8 Trainium2 NeuronCores.

Strategy (sharding_hint): shard the n_node dimension across the 8 cores.
 - adjacency COO entries partitioned by destination row; within a core the
   entries are grouped into 128-row windows, gathered from the (replicated /
   all-gathered) bf16 node table via dma_gather (one row -> one partition),
   and scatter-added through a one-hot matmul (one-hot built by
   gpsimd.local_scatter, values folded into the one-hot).
 - price/category tables + gate weights replicated; the big intra_gate
   contractions over n_node are sharded and finished with a small AllReduce.
 - item embeddings are all-gathered (bf16) between layers for the column
   gather in spmm / intra_gate.
All matmuls run in bf16 with fp32 PSUM accumulation; everything else fp32.
"""
import os
import sys
import math
import numpy as np
import ml_dtypes

sys.path.insert(0, '/opt/trn_rl_repo')

BF16 = ml_dtypes.bfloat16
P = 128          # partitions / EMB
EMB = 128


class Cfg:
    def __init__(self, n_node=50000, n_price=100, n_cat=500, ncore=8,
                 loh=32768, sg=12, gch=4):
        self.N = n_node
        self.NP = n_price
        self.NCAT = n_cat
        self.NCORE = ncore
        self.LOH = loh            # int16 gather-index limit (lo/hi table split)
        self.SG = sg              # blocks per local_scatter call (SG*128 <= 2047)
        self.GCH = gch            # blocks per dma_gather call
        self.NSH = n_node // ncore              # rows per core (assume divisible)
        assert self.NSH * ncore == n_node
        self.JT = math.ceil(self.NSH / P)       # 128-row j-tiles per shard
        self.NSHP = self.JT * P                 # padded shard rows (table layout)
        self.NTN = math.ceil(self.NSH / 512)    # node tiles of 512
        self.NPAD = self.NTN * 512              # padded node axis
        self.NW = self.NPAD // P                # adj windows (128 rows)
        self.NWR = self.JT                      # real windows (rest all-pad)
        self.NCATP = math.ceil(self.NCAT / P) * P
        self.NQC = self.NCATP // P              # cate chunks
        self.NTABP = self.NSHP * ncore          # padded global table rows


def _bf(x):
    return np.asarray(x, np.float32).astype(BF16)


def _wrap_idx(idx):
    """int16 stream -> dma_gather layout (128, len/16): slot i at [i%16, i//16],
    replicated across the 8 q7 cores."""
    idx = np.asarray(idx, np.int16)
    assert len(idx) % 16 == 0
    w = idx.reshape(-1, 16).T          # (16, L/16)
    return np.tile(w, (8, 1)).copy()   # (128, L/16)


def prep(inputs, cfg: Cfg):
    """Host-side sharding/layout prep. Only index manipulation, dtype casts,
    transposes and replication happen here - all arithmetic is on-device."""
    c = cfg
    f32 = np.float32
    emb = np.asarray(inputs['embedding'], f32)
    pri = np.asarray(inputs['pri_emb'], f32)
    cate = np.asarray(inputs['cate_emb'], f32)

    # -------- price/cate row permutations (positive mat_v first) --------
    m_pv = np.asarray(inputs['mat_pv'], f32)[:, 0]
    m_cv = np.asarray(inputs['mat_cv'], f32)[:, 0]
    perm_p = np.argsort(m_pv <= 0, kind='stable')
    perm_c = np.argsort(m_cv <= 0, kind='stable')
    npos_p = int((m_pv > 0).sum())
    npos_c = int((m_cv > 0).sum())
    inv_p = np.empty(c.NP, np.int64); inv_p[perm_p] = np.arange(c.NP)
    inv_c = np.empty(c.NCAT, np.int64); inv_c[perm_c] = np.arange(c.NCAT)

    m_pc = np.asarray(inputs['mat_pc'], f32)[:, 0][perm_p]   # per price row
    m_cp = np.asarray(inputs['mat_cp'], f32)[:, 0][perm_c]   # per cate row
    m_pv_s = m_pv[perm_p]
    m_cv_s = m_cv[perm_c]

    pri_s = pri[perm_p]
    cate_s = cate[perm_c]

    # -------- node table: padded layout (core section = NSHP rows) --------
    def pad_rows(a, rows):
        out = np.zeros((rows, a.shape[1]), a.dtype)
        out[:a.shape[0]] = a
        return out

    emb_tab = np.zeros((c.NTABP, EMB), BF16)
    for co in range(c.NCORE):
        emb_tab[co * c.NSHP: co * c.NSHP + c.NSH] = _bf(emb[co * c.NSH:(co + 1) * c.NSH])

    def padid(r):
        return (r // c.NSH) * c.NSHP + (r % c.NSH)

    # -------- adjacency: partition by destination row, window-group --------
    rows = np.asarray(inputs['adj_rows'], np.int64)
    cols = padid(np.asarray(inputs['adj_cols'], np.int64))
    vals = np.asarray(inputs['adj_vals'], f32)
    core_of = rows // c.NSH
    lr = rows % c.NSH
    win = lr // P
    wrow = lr % P
    is_lo = cols < c.LOH

    # counts per (core, window, side)
    cnt = np.zeros((c.NCORE, c.NW, 2), np.int64)
    np.add.at(cnt, (core_of, win, np.where(is_lo, 0, 1)), 1)
    nlo = np.ceil(cnt[:, :, 0].max(axis=0) / P).astype(np.int64)
    nhi = np.ceil(cnt[:, :, 1].max(axis=0) / P).astype(np.int64)

    # B-order block list
    blocks = []  # (w, side, slot_in_side_stream)
    locnt = hicnt = 0
    for w in range(c.NW):
        for _ in range(nlo[w]):
            blocks.append((w, 0, locnt)); locnt += 1
        for _ in range(nhi[w]):
            blocks.append((w, 1, hicnt)); hicnt += 1
    nblk_real = len(blocks)

    # scatter calls: SG-block groups, padded to even block count with dummies
    blocks_p = []
    scalls = []   # (k0, nb)
    for s in range(0, nblk_real, c.SG):
        grp = list(blocks[s:s + c.SG])
        if len(grp) % 2:
            grp.append((None, None, None))
        scalls.append((len(blocks_p), len(grp)))
        blocks_p.extend(grp)
    nblkp = len(blocks_p)

    glo_calls = [(s, min(c.GCH, locnt - s)) for s in range(0, locnt, c.GCH)]
    ghi_calls = [(s, min(c.GCH, hicnt - s)) for s in range(0, hicnt, c.GCH)]

    # per-core streams
    per_core = []
    order = np.lexsort((wrow, win, core_of))
    cols_o, vals_o, core_o, win_o, wrow_o, islo_o = (
        cols[order], vals[order], core_of[order], win[order], wrow[order], is_lo[order])
    for co in range(c.NCORE):
        sel = core_o == co
        cw, vv, ww, wr, lo = cols_o[sel], vals_o[sel], win_o[sel], wrow_o[sel], islo_o[sel]
        lo_idx = np.zeros(locnt * P, np.int16)
        hi_idx = np.zeros(hicnt * P, np.int16)
        s_lr = np.full((P, nblkp), -1.0, f32)
        s_v32 = np.zeros((P, nblkp), f32)
        locur = hicur = 0
        for w in range(c.NW):
            wm = ww == w
            for side, n_side, cur in ((0, nlo[w], locur), (1, nhi[w], hicur)):
                sm = wm & (lo if side == 0 else ~lo)
                cw_s, vv_s, wr_s = cw[sm], vv[sm], wr[sm]
                ne = len(cw_s)
                assert ne <= n_side * P
                stream = lo_idx if side == 0 else hi_idx
                base = cur * P
                stream[base:base + ne] = (cw_s - (0 if side == 0 else c.LOH)).astype(np.int16)
                if side == 0:
                    locur += n_side
                else:
                    hicur += n_side
        # fill scatter idx/val arrays walking padded blocks
        # (entry q of (w,side) stream maps to block slot q//128, partition q%128)
        # rebuild per (w, side) entry arrays:
        ptr = {}
        for w in range(c.NW):
            wm = ww == w
            for side in (0, 1):
                sm = wm & (lo if side == 0 else ~lo)
                ptr[(w, side)] = (vv[sm], wr[sm])
        side_pos = {}
        for kp, (w, side, slot) in enumerate(blocks_p):
            if w is None:
                continue
            vv_s, wr_s = ptr[(w, side)]
            j = side_pos.get((w, side), 0)
            lo_e = j * P
            hi_e = min(lo_e + P, len(vv_s))
            n = hi_e - lo_e
            if n > 0:
                s_lr[:n, kp] = wr_s[lo_e:hi_e].astype(f32)
                s_v32[:n, kp] = vv_s[lo_e:hi_e]
            side_pos[(w, side)] = j + 1
        per_core.append(dict(lo_idx=lo_idx, hi_idx=hi_idx, s_lr=s_lr, s_v32=s_v32))

    # -------- vp / vc scattered-A streams --------
    def build_A(rows_a, cols_a, vals_a, nchunk):
        rows_a = np.asarray(rows_a, np.int64)
        cols_a = np.asarray(cols_a, np.int64)
        vals_a = np.asarray(vals_a, f32)
        # coalesce duplicate (row, col) entries (sparse-matrix canonicalization;
        # local_scatter cannot express two writes to one A cell)
        ncols = int(cols_a.max()) + 1 if len(cols_a) else 1
        key0 = rows_a * ncols + cols_a
        uk, inv = np.unique(key0, return_inverse=True)
        vsum = np.zeros(len(uk), f32)
        np.add.at(vsum, inv, vals_a)
        rows_a, cols_a, vals_a = uk // ncols, uk % ncols, vsum
        core_a = rows_a // c.NSH
        pos_a = rows_a % c.NSH
        t_a = pos_a // 512
        rel_a = pos_a % 512
        q_a = cols_a // P
        p_a = cols_a % P
        # K = max entries per (core,tile,chunk,partition)
        key = (((core_a * c.NTN + t_a) * nchunk + q_a) * P + p_a)
        cnts = np.bincount(key, minlength=c.NCORE * c.NTN * nchunk * P)
        K = int(cnts.max())
        K = max(2, K + (K % 2))
        A_all = []
        for co in range(c.NCORE):
            A = np.zeros((c.NTN, nchunk, P, 512), BF16)
            sel = core_a == co
            tt, qq, pp, rr, vv2 = t_a[sel], q_a[sel], p_a[sel], rel_a[sel], vals_a[sel]
            A[tt, qq, pp, rr] = vv2.astype(BF16)
            A_all.append(A)
        return K, A_all

    vp_cols_p = inv_p[np.asarray(inputs['vp_cols'], np.int64)]
    vc_cols_c = inv_c[np.asarray(inputs['vc_cols'], np.int64)]
    KVP, vpA = build_A(inputs['vp_rows'], vp_cols_p, inputs['vp_vals'], 1)
    KVC, vcA = build_A(inputs['vc_rows'], vc_cols_c, inputs['vc_vals'], c.NQC)

    # -------- masks (transposed, permuted, padded, bf16) --------
    pvT = np.asarray(inputs['pv_dense'], f32).T[:, perm_p]       # (N, NP)
    cvT = np.asarray(inputs['cv_dense'], f32).T[:, perm_c]       # (N, NCAT)
    pcT = np.asarray(inputs['pc_dense'], f32).T[:, perm_p][perm_c, :]  # (NCAT, NP) rows cate
    cpT = np.asarray(inputs['cp_dense'], f32).T[:, perm_c][perm_p, :]  # (NP, NCAT) rows price
    maskL1 = []
    maskL2 = []
    for co in range(c.NCORE):
        sl = slice(co * c.NSH, (co + 1) * c.NSH)
        big = np.zeros((c.NSHP, c.NP + c.NCAT), BF16)
        big[:c.NSH, :c.NP] = _bf(pvT[sl])
        big[:c.NSH, c.NP:] = _bf(cvT[sl])
        maskL1.append(big)
        maskL2.append(np.ascontiguousarray(big[:, :c.NP]))
    pcT_m = np.zeros((c.NCATP, c.NP), BF16); pcT_m[:c.NCAT] = _bf(pcT)
    cpT_m = np.zeros((P, c.NCAT), BF16); cpT_m[:c.NP] = _bf(cpT)

    # -------- tables / weights / broadcast vectors --------
    W = {k: np.asarray(inputs[k], f32) for k in
         ['Wai', 'bai', 'Wi1', 'bi1', 'Wi2', 'bi2', 'Wap', 'bap', 'Wp1', 'bp1',
          'Wp2', 'bp2', 'Wac', 'bac', 'Wc1', 'bc1', 'Wc2', 'bc2']}
    def chunks(Wa):
        WaT = Wa.T  # (3*EMB, EMB)
        return [WaT[k * P:(k + 1) * P, :] for k in range(3)]
    Wstack = np.stack(chunks(W['Wai']) + [W['Wi1'].T, W['Wi2'].T]
                      + chunks(W['Wap']) + [W['Wp1'].T, W['Wp2'].T]
                      + chunks(W['Wac']) + [W['Wc1'].T, W['Wc2'].T]).astype(BF16)
    Bstack = np.stack([
        np.stack([W['bai'], W['bi1'], W['bi2']], axis=1),
        np.stack([W['bap'], W['bp1'], W['bp2']], axis=1),
        np.stack([W['bac'], W['bc1'], W['bc2']], axis=1)]).astype(f32)  # (3,128,3)

    pri_tab = np.zeros((P, EMB), BF16); pri_tab[:c.NP] = _bf(pri_s)
    cate_tab = np.zeros((c.NCATP, EMB), BF16); cate_tab[:c.NCAT] = _bf(cate_s)

    mstack = np.zeros((P, 2 * (c.NP + c.NCAT)), f32)
    o = 0
    for v in (m_pv_s, m_cv_s, m_pc, m_cp):
        mstack[:, o:o + len(v)] = v[None, :]
        o += len(v)

    rep = dict(
        emb_tab=emb_tab, pri_tab=pri_tab, cate_tab=cate_tab,
        priT32=np.ascontiguousarray(pri_s.T), priT16=np.ascontiguousarray(_bf(pri_s.T)),
        cateT32=np.ascontiguousarray(cate_s.T), cateT16=np.ascontiguousarray(_bf(cate_s.T)),
        Wstack=Wstack, Bstack=Bstack, mstack=mstack,
        pcT_m=pcT_m, cpT_m=cpT_m,
        ident=np.eye(P, dtype=BF16), ones_c=np.ones((P, 1), BF16),
        ones_r=np.ones((1, P), f32),
        iota_r=np.tile(np.arange(P, dtype=f32)[None, :], (P, 1)),
    )

    in_maps = []
    for co in range(c.NCORE):
        sl = slice(co * c.NSH, (co + 1) * c.NSH)
        embT32 = np.zeros((P, c.NPAD), f32)
        embT32[:, :c.NSH] = emb[sl].T
        emb_norm = np.zeros((c.NSHP, EMB), BF16)
        emb_norm[:c.NSH] = _bf(emb[sl])
        m = dict(rep)
        m.update(
            embT32=embT32, embT16=embT32.astype(BF16),
            emb_norm=emb_norm,
            maskL1=maskL1[co], maskL2=maskL2[co],
            glo_idx=_wrap_idx(per_core[co]['lo_idx']),
            ghi_idx=_wrap_idx(per_core[co]['hi_idx']),
            s_lr=per_core[co]['s_lr'], s_v32=per_core[co]['s_v32'],
            vpA=vpA[co], vcA=vcA[co],
        )
        in_maps.append(m)

    meta = dict(nlo=nlo.tolist(), nhi=nhi.tolist(), blocks_p=blocks_p,
                scalls=scalls, glo_calls=glo_calls, ghi_calls=ghi_calls,
                locnt=locnt, hicnt=hicnt, nblkp=nblkp,
                KVP=KVP, KVC=KVC, npos_p=npos_p, npos_c=npos_c,
                perm_p=perm_p, perm_c=perm_c,
                layers=int(np.asarray(inputs['layers'])))
    return in_maps, meta


# ---------------------------------------------------------------------------

def build(cfg: Cfg, meta):
    import concourse.bacc as bacc
    import concourse.bass as bass
    import concourse.tile as tile
    import concourse.mybir as mybir
    import concourse.bass_isa as bass_isa

    c = cfg
    dt = mybir.dt
    ALU = mybir.AluOpType
    ACTF = mybir.ActivationFunctionType
    AX = mybir.AxisListType
    layers = meta['layers']
    RG = [list(range(c.NCORE))]
    NP_, NCAT, NQC = c.NP, c.NCAT, c.NQC
    RC = NP_ + NCAT

    nc = bacc.Bacc("TRN2", target_bir_lowering=False, debug=False,
                   num_devices=c.NCORE)

    def din(name, shape, dtype):
        return nc.dram_tensor(name, list(shape), dtype, kind="ExternalInput")

    # ---- inputs ----
    emb_tab = din('emb_tab', (c.NTABP, EMB), dt.bfloat16)
    pri_tab_i = din('pri_tab', (P, EMB), dt.bfloat16)
    cate_tab_i = din('cate_tab', (c.NCATP, EMB), dt.bfloat16)
    priT32_i = din('priT32', (EMB, NP_), dt.float32)
    priT16_i = din('priT16', (EMB, NP_), dt.bfloat16)
    cateT32_i = din('cateT32', (EMB, NCAT), dt.float32)
    cateT16_i = din('cateT16', (EMB, NCAT), dt.bfloat16)
    Wstack = din('Wstack', (15, P, EMB), dt.bfloat16)
    Bstack = din('Bstack', (3, P, 3), dt.float32)
    mstack = din('mstack', (P, 2 * RC), dt.float32)
    pcT_m = din('pcT_m', (c.NCATP, NP_), dt.bfloat16)
    cpT_m = din('cpT_m', (P, NCAT), dt.bfloat16)
    ident_i = din('ident', (P, P), dt.bfloat16)
    ones_c_i = din('ones_c', (P, 1), dt.bfloat16)
    ones_r_i = din('ones_r', (1, P), dt.float32)
    embT32_i = din('embT32', (P, c.NPAD), dt.float32)
    embT16_i = din('embT16', (P, c.NPAD), dt.bfloat16)
    emb_norm_i = din('emb_norm', (c.NSHP, EMB), dt.bfloat16)
    maskL1_i = din('maskL1', (c.NSHP, RC), dt.bfloat16)
    maskL2_i = din('maskL2', (c.NSHP, NP_), dt.bfloat16)
    glo_i = din('glo_idx', (P, max(16, meta['locnt'] * P // 16)), dt.int16)
    ghi_i = din('ghi_idx', (P, max(16, meta['hicnt'] * P // 16)), dt.int16)
    s_lr_i = din('s_lr', (P, meta['nblkp']), dt.float32)
    s_v32_i = din('s_v32', (P, meta['nblkp']), dt.float32)
    vpA_i = din('vpA', (c.NTN, 1, P, 512), dt.bfloat16)
    vcA_i = din('vcA', (c.NTN, NQC, P, 512), dt.bfloat16)
    iota_i = din('iota_r', (P, P), dt.float32)

    item_t = nc.dram_tensor('item_t', [P, c.NSH], dt.float32, kind="ExternalOutput")
    price_t = nc.dram_tensor('price_t', [P, NP_], dt.float32, kind="ExternalOutput")

    # per-layer collective tensors
    ccmax_in = [nc.dram_tensor(f'ccmax_in{L}', [1, 8], dt.float32) for L in range(layers)]
    ccmax_out = [nc.dram_tensor(f'ccmax_out{L}', [1, 8], dt.float32, addr_space="Shared")
                 for L in range(layers)]
    arp_in, arp_out = [], []
    for L in range(layers):
        rc = RC if L < layers - 1 else NP_
        arp_in.append(nc.dram_tensor(f'arp_in{L}', [P + 1, rc], dt.float32))
        arp_out.append(nc.dram_tensor(f'arp_out{L}', [P + 1, rc], dt.float32,
                                      addr_space="Shared"))
    ag_in = [nc.dram_tensor(f'ag_in{L}', [c.NSHP, EMB], dt.bfloat16)
             for L in range(max(0, layers - 1))]
    ag_out = [nc.dram_tensor(f'ag_out{L}', [c.NTABP, EMB], dt.bfloat16,
                             addr_space="Shared")
              for L in range(max(0, layers - 1))]
    itemT32_d = [nc.dram_tensor(f'itemT32_{L}', [P, c.NPAD], dt.float32)
                 for L in range(max(0, layers - 1))]
    itemT16_d = [nc.dram_tensor(f'itemT16_{L}', [P, c.NPAD], dt.bfloat16)
                 for L in range(max(0, layers - 1))]

    nlo, nhi = meta['nlo'], meta['nhi']
    blocks_p, scalls = meta['blocks_p'], meta['scalls']
    glo_calls, ghi_calls = meta['glo_calls'], meta['ghi_calls']

    from contextlib import ExitStack
    with tile.TileContext(nc) as tc, ExitStack() as es:
        cp_ = es.enter_context(tc.tile_pool(name="const", bufs=1))
        sp = es.enter_context(tc.tile_pool(name="stream", bufs=1))
        pp = es.enter_context(tc.tile_pool(name="ps", bufs=1, space="PSUM"))

        def dma(out_ap, in_ap):
            nc.sync.dma_start(out_ap, in_ap)

        # ---------------- constants ----------------
        W_sb = cp_.tile([P, 15, EMB], dt.bfloat16)
        dma(W_sb[:], Wstack[:, :, :].rearrange("i p q -> p i q"))
        bias3 = cp_.tile([P, 3, 3], dt.float32)
        dma(bias3[:], Bstack[:, :, :].rearrange("i p q -> p i q"))
        bcomb = cp_.tile([P, 3], dt.float32)
        for br in range(3):
            nc.vector.tensor_reduce(bcomb[:, br:br + 1], bias3[:, br, :],
                                    axis=AX.X, op=ALU.add)
        ident = cp_.tile([P, P], dt.bfloat16); dma(ident[:], ident_i[:, :])
        ones_c = cp_.tile([P, 1], dt.bfloat16); dma(ones_c[:], ones_c_i[:, :])
        ones_r = cp_.tile([1, P], dt.float32); dma(ones_r[:], ones_r_i[:, :])
        m_sb = cp_.tile([P, 2 * RC], dt.float32); dma(m_sb[:], mstack[:, :])
        m_pv_b = m_sb[:, 0:NP_]
        m_cv_b = m_sb[:, NP_:RC]
        m_pc_b = m_sb[:, RC:RC + NP_]
        m_cp_b = m_sb[:, RC + NP_:2 * RC]
        pcT_sb = cp_.tile([P, NQC, NP_], dt.bfloat16)
        dma(pcT_sb[:], pcT_m[:, :].rearrange("(q p) r -> p q r", p=P))
        cpT_sb = cp_.tile([P, NCAT], dt.bfloat16); dma(cpT_sb[:], cpT_m[:, :])
        pri_norm0 = cp_.tile([P, EMB], dt.bfloat16); dma(pri_norm0[:], pri_tab_i[:, :])
        cate_norm0 = cp_.tile([P, NQC, EMB], dt.bfloat16)
        dma(cate_norm0[:], cate_tab_i[:, :].rearrange("(q p) e -> p q e", p=P))
        priT32_0 = cp_.tile([P, NP_], dt.float32); dma(priT32_0[:], priT32_i[:, :])
        priT16_0 = cp_.tile([P, NP_], dt.bfloat16); dma(priT16_0[:], priT16_i[:, :])
        cateT32_0 = cp_.tile([P, NCAT], dt.float32); dma(cateT32_0[:], cateT32_i[:, :])
        cateT16_0 = cp_.tile([P, NCAT], dt.bfloat16); dma(cateT16_0[:], cateT16_i[:, :])
        glo_sb = cp_.tile([P, glo_i.shape[1]], dt.int16); dma(glo_sb[:], glo_i[:, :])
        ghi_sb = cp_.tile([P, ghi_i.shape[1]], dt.int16); dma(ghi_sb[:], ghi_i[:, :])
        s_lr_sb = cp_.tile([P, meta['nblkp']], dt.float32); dma(s_lr_sb[:], s_lr_i[:, :])
        s_v32_sb = cp_.tile([P, meta['nblkp']], dt.float32); dma(s_v32_sb[:], s_v32_i[:, :])
        iota_sb = cp_.tile([P, P], dt.float32); dma(iota_sb[:], iota_i[:, :])

        # e1 / norm-tile sources stay in DRAM and are streamed per tile
        state = dict(e1d32=embT32_i, e1d16=embT16_i, normd=emb_norm_i,
                     priT32=priT32_0, priT16=priT16_0, pri_norm=pri_norm0,
                     cateT32=cateT32_0, cateT16=cateT16_0, cate_norm=cate_norm0,
                     tab=emb_tab)

        for L in range(layers):
            last = (L == layers - 1)
            rc = RC if not last else NP_
            mask_i = maskL1_i if not last else maskL2_i

            # ================= phase S: stats =================
            s_col = sp.tile([P, c.JT], dt.float32, tag="s_col", bufs=2)
            for j in range(c.JT):
                nrm_s = sp.tile([P, EMB], dt.bfloat16, tag="nrm_s", bufs=3)
                dma(nrm_s[:], state['normd'][j * P:(j + 1) * P, :])
                nc.vector.tensor_reduce(s_col[:, j:j + 1], nrm_s[:],
                                        axis=AX.X, op=ALU.add)
            smax_l = sp.tile([P, 1], dt.float32, tag="st1", bufs=4)
            nc.vector.tensor_reduce(smax_l[:], s_col[:], axis=AX.X, op=ALU.max)
            nc.gpsimd.partition_all_reduce(smax_l[:], smax_l[:], 128,
                                           bass_isa.ReduceOp.max)
            smin_l = sp.tile([P, 1], dt.float32, tag="st1", bufs=4)
            nc.vector.tensor_reduce(smin_l[:], s_col[:], axis=AX.X, op=ALU.min)
            nc.vector.tensor_scalar_mul(smin_l[:], smin_l[:], -1.0)
            nc.gpsimd.partition_all_reduce(smin_l[:], smin_l[:], 128,
                                           bass_isa.ReduceOp.max)
            stage = sp.tile([1, 8], dt.float32, tag="st8", bufs=2)
            nc.vector.memset(stage[:], 0.0)
            nc.vector.tensor_copy(stage[0:1, 0:1], smax_l[0:1, :])
            nc.vector.tensor_copy(stage[0:1, 1:2], smin_l[0:1, :])
            dma(ccmax_in[L][:, :], stage[:])
            nc.gpsimd.collective_compute(
                "AllReduce", ALU.max, replica_groups=RG,
                ins=[ccmax_in[L][:, :].opt()], outs=[ccmax_out[L][:, :].opt()])
            strow = sp.tile([1, 8], dt.float32, tag="st8", bufs=2)
            dma(strow[:], ccmax_out[L][:, :])
            stbc = sp.tile([P, 8], dt.float32, tag="stbc", bufs=2)
            nc.gpsimd.partition_broadcast(stbc[:], strow[:])
            smax_bc = stbc[:, 0:1]
            smin_bc = sp.tile([P, 1], dt.float32, tag="st1", bufs=4)
            nc.vector.tensor_scalar_mul(smin_bc[:], stbc[:, 1:2], -1.0)
            SSmax = sp.tile([P, c.JT], dt.float32, tag="ssm", bufs=2)
            nc.vector.tensor_scalar(SSmax[:], s_col[:], smax_bc, None, op0=ALU.subtract)
            SSmin = sp.tile([P, c.JT], dt.float32, tag="ssn", bufs=2)
            nc.vector.tensor_scalar(SSmin[:], s_col[:], smin_bc[:], None, op0=ALU.subtract)

            # local stats for pc (cate table) and cp (price table)
            s_cate = sp.tile([P, NQC], dt.float32, tag="s_cate", bufs=2)
            for q in range(NQC):
                nc.vector.tensor_reduce(s_cate[:, q:q + 1], state['cate_norm'][:, q, :],
                                        axis=AX.X, op=ALU.add)
            cmax = sp.tile([P, 1], dt.float32, tag="st1", bufs=4)
            nc.vector.tensor_reduce(cmax[:], s_cate[:], axis=AX.X, op=ALU.max)
            nc.gpsimd.partition_all_reduce(cmax[:], cmax[:], 128, bass_isa.ReduceOp.max)
            cmin = sp.tile([P, 1], dt.float32, tag="st1", bufs=4)
            nc.vector.tensor_reduce(cmin[:], s_cate[:], axis=AX.X, op=ALU.min)
            nc.vector.tensor_scalar_mul(cmin[:], cmin[:], -1.0)
            nc.gpsimd.partition_all_reduce(cmin[:], cmin[:], 128, bass_isa.ReduceOp.max)
            nc.vector.tensor_scalar_mul(cmin[:], cmin[:], -1.0)
            c_pc = sp.tile([P, NP_], dt.float32, tag="c_pc", bufs=2)
            t1 = sp.tile([P, NCAT], dt.float32, tag="ctmp", bufs=2)
            nc.vector.tensor_scalar(c_pc[:], m_pc_b, cmax[:], None, op0=ALU.mult)
            nc.vector.tensor_scalar(t1[:, :NP_], m_pc_b, cmin[:], None, op0=ALU.mult)
            nc.vector.tensor_max(c_pc[:], c_pc[:], t1[:, :NP_])
            c_cp = None
            if not last:
                s_pri = sp.tile([P, 1], dt.float32, tag="st1", bufs=4)
                nc.vector.tensor_reduce(s_pri[:], state['pri_norm'][:], axis=AX.X, op=ALU.add)
                pmax = sp.tile([P, 1], dt.float32, tag="st1", bufs=4)
                nc.vector.tensor_copy(pmax[:], s_pri[:])
                nc.gpsimd.partition_all_reduce(pmax[:], pmax[:], 128, bass_isa.ReduceOp.max)
                pmin = sp.tile([P, 1], dt.float32, tag="st1", bufs=4)
                nc.vector.tensor_scalar_mul(pmin[:], s_pri[:], -1.0)
                nc.gpsimd.partition_all_reduce(pmin[:], pmin[:], 128, bass_isa.ReduceOp.max)
                nc.vector.tensor_scalar_mul(pmin[:], pmin[:], -1.0)
                c_cp = sp.tile([P, NCAT], dt.float32, tag="c_cp", bufs=2)
                nc.vector.tensor_scalar(c_cp[:], m_cp_b, pmax[:], None, op0=ALU.mult)
                nc.vector.tensor_scalar(t1[:], m_cp_b, pmin[:], None, op0=ALU.mult)
                nc.vector.tensor_max(c_cp[:], c_cp[:], t1[:])
                # s_pri per-partition scalars for cp intra (j = price rows)
                state['s_pri'] = s_pri
            state['s_cate'] = s_cate

            # ================= phase A: spmm + item gate =================
            gcall_done = [0, 0]   # lo, hi calls issued
            g_tiles = [{}, {}]
            scall_done = 0
            s_tiles = {}

            def issue_gather(side, callidx):
                calls = glo_calls if side == 0 else ghi_calls
                s0, nb = calls[callidx]
                gt = sp.tile([P, c.GCH, EMB], dt.bfloat16,
                             tag=f"g{side}", bufs=2)
                idxs = (glo_sb if side == 0 else ghi_sb)
                src = state['tab'][:, :] if side == 0 else state['tab'][c.LOH:, :]
                nc.gpsimd.dma_gather(
                    out_ap=gt[:, 0:nb, :],
                    in_ap=src,
                    idxs_ap=idxs[:, s0 * 8: s0 * 8 + nb * 8],
                    num_idxs=nb * P, num_idxs_reg=nb * P,
                    elem_size=EMB)
                g_tiles[side][callidx] = gt

            def issue_scatter(callidx):
                k0, nb = scalls[callidx]
                st = sp.tile([P, c.SG * P], dt.bfloat16, tag="sblk", bufs=3)
                for rel in range(nb):
                    kp = k0 + rel
                    if blocks_p[kp][0] is None:
                        continue
                    nc.vector.scalar_tensor_tensor(
                        st[:, rel * P:(rel + 1) * P], iota_sb[:],
                        s_lr_sb[:, kp:kp + 1],
                        s_v32_sb[:, kp:kp + 1].to_broadcast([P, P]),
                        op0=ALU.is_equal, op1=ALU.mult)
                s_tiles[callidx] = st

            kptr = 0   # walks blocks_p
            for t in range(c.NTN):
                # ---- e2 (vp) / e3 (vc) ----
                a_vp = sp.tile([P, 512], dt.bfloat16, tag="avp", bufs=2)
                dma(a_vp[:], vpA_i[t, 0, :, :])
                e2_ps = pp.tile([P, 512], dt.float32, tag="e2ps")
                nc.tensor.matmul(e2_ps[:], state['pri_norm'][:], a_vp[:],
                                 start=True, stop=True)
                e3_ps = pp.tile([P, 512], dt.float32, tag="e3ps")
                for q in range(NQC):
                    a_vc = sp.tile([P, 512], dt.bfloat16, tag="avc", bufs=2)
                    dma(a_vc[:], vcA_i[t, q, :, :])
                    nc.tensor.matmul(e3_ps[:], state['cate_norm'][:, q, :], a_vc[:],
                                     start=(q == 0), stop=(q == NQC - 1))
                e2_16 = sp.tile([P, 512], dt.bfloat16, tag="e2_16", bufs=2)
                nc.scalar.copy(e2_16[:], e2_ps[:])
                e3_16 = sp.tile([P, 512], dt.bfloat16, tag="e3_16", bufs=2)
                nc.scalar.copy(e3_16[:], e3_ps[:])

                # ---- adj windows ----
                adj_ps = pp.tile([P, 512], dt.float32, tag="adjps")
                for wl in range(4):
                    w = t * 4 + wl
                    wblocks = [(kp, blk) for kp, blk in enumerate(blocks_p)
                               if blk[0] == w]
                    if not wblocks:
                        nc.vector.memset(adj_ps[:, wl * P:(wl + 1) * P], 0.0)
                        continue
                    nb_w = len(wblocks)
                    for bi, (kp, (bw, side, slot)) in enumerate(wblocks):
                        gcall = slot // c.GCH
                        brel = slot % c.GCH
                        while gcall_done[side] <= gcall:
                            issue_gather(side, gcall_done[side])
                            gcall_done[side] += 1
                        sc = next(i for i, (k0, nb) in enumerate(scalls)
                                  if k0 <= kp < k0 + nb)
                        while scall_done <= sc:
                            issue_scatter(scall_done)
                            scall_done += 1
                        k0 = scalls[sc][0]
                        gt = g_tiles[side][gcall]
                        st = s_tiles[sc]
                        nc.tensor.matmul(
                            adj_ps[:, wl * P:(wl + 1) * P],
                            gt[:, brel, :],
                            st[:, (kp - k0) * P:(kp - k0 + 1) * P],
                            start=(bi == 0), stop=(bi == nb_w - 1))

                # ---- gate ----
                gate_ps = pp.tile([P, 512], dt.float32, tag="gatetr")
                nsl = slice(t * 512, (t + 1) * 512)
                e1t16 = sp.tile([P, 512], dt.bfloat16, tag="e1t16", bufs=2)
                dma(e1t16[:], state['e1d16'][:, nsl])
                rhs_list = [e1t16[:], e2_16[:], e3_16[:], e2_16[:], e3_16[:]]
                for i5 in range(5):
                    nc.tensor.matmul(gate_ps[:], W_sb[:, i5, :], rhs_list[i5],
                                     start=(i5 == 0), stop=(i5 == 4))
                g_sb = sp.tile([P, 512], dt.float32, tag="g_sb", bufs=2)
                nc.scalar.activation(g_sb[:], gate_ps[:], ACTF.Sigmoid,
                                     bias=bcomb[:, 0:1])
                # ---- combine: item = e1 + e3 + g*(e2-e3) + adj ----
                e1t32 = sp.tile([P, 512], dt.float32, tag="e1t32", bufs=2)
                dma(e1t32[:], state['e1d32'][:, nsl])
                e2_32 = sp.tile([P, 512], dt.float32, tag="e2_32", bufs=2)
                nc.scalar.copy(e2_32[:], e2_ps[:])
                x = sp.tile([P, 512], dt.float32, tag="xcmb", bufs=2)
                nc.vector.tensor_sub(x[:], e2_32[:], e3_ps[:])
                nc.vector.tensor_mul(x[:], x[:], g_sb[:])
                nc.vector.tensor_add(x[:], x[:], e1t32[:])
                nc.vector.tensor_add(x[:], x[:], e3_ps[:])
                xo = sp.tile([P, 512], dt.float32, tag="xout", bufs=2)
                nc.vector.tensor_add(xo[:], x[:], adj_ps[:])
                if not last:
                    dma(itemT32_d[L][:, nsl], xo[:])
                    it16 = sp.tile([P, 512], dt.bfloat16, tag="it16", bufs=2)
                    nc.scalar.copy(it16[:], xo[:])
                    dma(itemT16_d[L][:, nsl], it16[:])
                    normt = sp.tile([P, 4, EMB], dt.bfloat16, tag="normt", bufs=2)
                    for q4 in range(4):
                        j = t * 4 + q4
                        if j >= c.JT:
                            continue
                        tr_ps = pp.tile([P, P], dt.bfloat16, tag="gatetr")
                        nc.tensor.transpose(tr_ps[:],
                                            it16[:, q4 * P:(q4 + 1) * P], ident[:])
                        nc.vector.tensor_copy(normt[:, q4, :], tr_ps[:])
                        dma(ag_in[L][j * P:(j + 1) * P, :], normt[:, q4, :])
                else:
                    lo_n = t * 512
                    hi_n = min((t + 1) * 512, c.NSH)
                    if hi_n > lo_n:
                        dma(item_t[:, lo_n:hi_n], xo[:, 0:hi_n - lo_n])

            if not last:
                nc.gpsimd.collective_compute(
                    "AllGather", ALU.bypass, replica_groups=RG,
                    ins=[ag_in[L][:, :].opt()], outs=[ag_out[L][:, :].opt()])

            # ================= phase B: pv (+cv) contraction =================
            pv_ps = pp.tile([P, NP_], dt.float32, tag="acc1")
            dpv_ps = pp.tile([1, NP_], dt.float32, tag="d1")
            cv_ps = dcv_ps = None
            if not last:
                cv_ps = pp.tile([P, NCAT], dt.float32, tag="acc2")
                dcv_ps = pp.tile([1, NCAT], dt.float32, tag="d2")
            npos_p, npos_c = meta['npos_p'], meta['npos_c']
            for j in range(c.JT):
                mk = sp.tile([P, rc], dt.bfloat16, tag="mask", bufs=3)
                dma(mk[:], mask_i[j * P:(j + 1) * P, :])
                nrm_b = sp.tile([P, EMB], dt.bfloat16, tag="nrm_b", bufs=3)
                dma(nrm_b[:], state['normd'][j * P:(j + 1) * P, :])
                tf = sp.tile([P, rc], dt.float32, tag="tf", bufs=2)
                if npos_p:
                    nc.vector.tensor_scalar(tf[:, 0:npos_p], m_pv_b[:, 0:npos_p],
                                            SSmax[:, j:j + 1], None, op0=ALU.mult)
                if npos_p < NP_:
                    nc.vector.tensor_scalar(tf[:, npos_p:NP_], m_pv_b[:, npos_p:NP_],
                                            SSmin[:, j:j + 1], None, op0=ALU.mult)
                if not last:
                    if npos_c:
                        nc.vector.tensor_scalar(tf[:, NP_:NP_ + npos_c],
                                                m_cv_b[:, 0:npos_c],
                                                SSmax[:, j:j + 1], None, op0=ALU.mult)
                    if npos_c < NCAT:
                        nc.vector.tensor_scalar(tf[:, NP_ + npos_c:],
                                                m_cv_b[:, npos_c:],
                                                SSmin[:, j:j + 1], None, op0=ALU.mult)
                eb = sp.tile([P, rc], dt.bfloat16, tag="eb", bufs=2)
                nc.scalar.activation(eb[:], tf[:], ACTF.Exp)
                em = sp.tile([P, rc], dt.bfloat16, tag="em", bufs=2)
                nc.vector.tensor_mul(em[:], eb[:], mk[:])
                st_, sp_ = (j == 0), (j == c.JT - 1)
                nc.tensor.matmul(pv_ps[:], nrm_b[:], em[:, 0:NP_],
                                 start=st_, stop=sp_)
                nc.tensor.matmul(dpv_ps[:], ones_c[:], em[:, 0:NP_],
                                 start=st_, stop=sp_)
                if not last:
                    nc.tensor.matmul(cv_ps[:], nrm_b[:], em[:, NP_:],
                                     start=st_, stop=sp_)
                    nc.tensor.matmul(dcv_ps[:], ones_c[:], em[:, NP_:],
                                     start=st_, stop=sp_)
            nums = sp.tile([P, rc], dt.float32, tag="nums", bufs=2)
            nc.scalar.copy(nums[:, 0:NP_], pv_ps[:])
            dens = sp.tile([1, rc], dt.float32, tag="dens", bufs=2)
            nc.vector.tensor_copy(dens[0:1, 0:NP_], dpv_ps[:])
            if not last:
                nc.scalar.copy(nums[:, NP_:], cv_ps[:])
                nc.vector.tensor_copy(dens[0:1, NP_:], dcv_ps[:])
            dma(arp_in[L][0:P, :], nums[:])
            dma(arp_in[L][P:P + 1, :], dens[:])
            nc.gpsimd.collective_compute(
                "AllReduce", ALU.add, replica_groups=RG,
                ins=[arp_in[L][:, :].opt()], outs=[arp_out[L][:, :].opt()])
            numsR = sp.tile([P, rc], dt.float32, tag="numsR", bufs=2)
            dma(numsR[:], arp_out[L][0:P, :])
            densR = sp.tile([1, rc], dt.float32, tag="densR", bufs=2)
            dma(densR[:], arp_out[L][P:P + 1, :])
            recip = sp.tile([1, rc], dt.float32, tag="recip", bufs=2)
            nc.vector.reciprocal(recip[:], densR[:])
            e2pT32 = sp.tile([P, NP_], dt.float32, tag="e2pT32", bufs=2)
            e2pT16 = sp.tile([P, NP_], dt.bfloat16, tag="e2pT16", bufs=2)
            bc_ps = pp.tile([P, NP_], dt.float32, tag="d1")
            nc.tensor.matmul(bc_ps[:], ones_r[:], recip[0:1, 0:NP_], start=True, stop=True)
            nc.vector.tensor_mul(e2pT32[:], numsR[:, 0:NP_], bc_ps[:])
            nc.scalar.copy(e2pT16[:], e2pT32[:])
            if not last:
                e3cT32 = sp.tile([P, NCAT], dt.float32, tag="e3cT32", bufs=2)
                e3cT16 = sp.tile([P, NCAT], dt.bfloat16, tag="e3cT16", bufs=2)
                bc2_ps = pp.tile([P, NCAT], dt.float32, tag="d2")
                nc.tensor.matmul(bc2_ps[:], ones_r[:], recip[0:1, NP_:], start=True, stop=True)
                nc.vector.tensor_mul(e3cT32[:], numsR[:, NP_:], bc2_ps[:])
                nc.scalar.copy(e3cT16[:], e3cT32[:])

            # ================= phase C: pc/cp intra + price/cate gates =======
            # pc intra (j = cate rows): e3 of price branch
            pcn_ps = pp.tile([P, NP_], dt.float32, tag="acc1")
            pcdS_ps = pp.tile([1, NP_], dt.float32, tag="d1")
            pcdZ_ps = pp.tile([1, NP_], dt.float32, tag="d2")
            for q in range(NQC):
                tf2 = sp.tile([P, NP_], dt.float32, tag="tf2", bufs=2)
                nc.vector.tensor_scalar(tf2[:], m_pc_b, state['s_cate'][:, q:q + 1],
                                        None, op0=ALU.mult)
                nc.vector.tensor_sub(tf2[:], tf2[:], c_pc[:])
                eb2 = sp.tile([P, NP_], dt.bfloat16, tag="eb2", bufs=2)
                nc.scalar.activation(eb2[:], tf2[:], ACTF.Exp)
                em2 = sp.tile([P, NP_], dt.bfloat16, tag="em2", bufs=2)
                nc.vector.tensor_mul(em2[:], eb2[:], pcT_sb[:, q, :])
                st_, sp_ = (q == 0), (q == NQC - 1)
                nc.tensor.matmul(pcn_ps[:], state['cate_norm'][:, q, :], em2[:],
                                 start=st_, stop=sp_)
                nc.tensor.matmul(pcdS_ps[:], ones_c[:], em2[:], start=st_, stop=sp_)
                nc.tensor.matmul(pcdZ_ps[:], ones_c[:], eb2[:], start=st_, stop=sp_)
            dpc = sp.tile([1, NP_], dt.float32, tag="dpc", bufs=2)
            nc.vector.tensor_scalar(dpc[:], pcdZ_ps[:], 1e-8, None, op0=ALU.mult)
            nc.vector.tensor_add(dpc[:], dpc[:], pcdS_ps[:])
            nc.vector.reciprocal(dpc[:], dpc[:])
            bc3_ps = pp.tile([P, NP_], dt.float32, tag="d1")
            nc.tensor.matmul(bc3_ps[:], ones_r[:], dpc[:], start=True, stop=True)
            pcn_sb = sp.tile([P, NP_], dt.float32, tag="pcn_sb", bufs=2)
            nc.scalar.copy(pcn_sb[:], pcn_ps[:])
            e3pT32 = sp.tile([P, NP_], dt.float32, tag="e3pT32", bufs=2)
            nc.vector.tensor_mul(e3pT32[:], pcn_sb[:], bc3_ps[:])
            e3pT16 = sp.tile([P, NP_], dt.bfloat16, tag="e3pT16", bufs=2)
            nc.scalar.copy(e3pT16[:], e3pT32[:])

            # price gate
            pg_ps = pp.tile([P, NP_], dt.float32, tag="acc2")
            rhs5 = [state['priT16'][:], e2pT16[:], e3pT16[:], e2pT16[:], e3pT16[:]]
            for i5 in range(5):
                nc.tensor.matmul(pg_ps[:], W_sb[:, 5 + i5, :], rhs5[i5],
                                 start=(i5 == 0), stop=(i5 == 4))
            gp_sb = sp.tile([P, NP_], dt.float32, tag="gp_sb", bufs=2)
            nc.scalar.activation(gp_sb[:], pg_ps[:], ACTF.Sigmoid, bias=bcomb[:, 1:2])
            xp = sp.tile([P, NP_], dt.float32, tag="xp", bufs=2)
            nc.vector.tensor_sub(xp[:], e2pT32[:], e3pT32[:])
            nc.vector.tensor_mul(xp[:], xp[:], gp_sb[:])
            nc.vector.tensor_add(xp[:], xp[:], state['priT32'][:])
            nc.vector.tensor_add(xp[:], xp[:], e3pT32[:])
            if last:
                dma(price_t[:, :], xp[:])
            else:
                priT32n = cp_.tile([P, NP_], dt.float32, tag=f"priT32_{L % 2 + 1}")
                nc.vector.tensor_copy(priT32n[:], xp[:])
                priT16n = cp_.tile([P, NP_], dt.bfloat16, tag=f"priT16_{L % 2 + 1}")
                nc.scalar.copy(priT16n[:], xp[:])
                prn_ps = pp.tile([P, P], dt.bfloat16, tag="d1")
                nc.tensor.transpose(prn_ps[0:NP_, :], priT16n[:], ident[:])
                pri_normN = cp_.tile([P, EMB], dt.bfloat16, tag=f"pri_norm_{L % 2 + 1}")
                nc.vector.memset(pri_normN[:], 0.0)
                nc.vector.tensor_copy(pri_normN[0:NP_, :], prn_ps[0:NP_, :])

                # cp intra (j = price rows): e2 of cate branch
                tf3 = sp.tile([P, NCAT], dt.float32, tag="tf3", bufs=2)
                nc.vector.tensor_scalar(tf3[:], m_cp_b, state['s_pri'][:],
                                        None, op0=ALU.mult)
                nc.vector.tensor_sub(tf3[:], tf3[:], c_cp[:])
                eb3 = sp.tile([P, NCAT], dt.bfloat16, tag="eb3", bufs=2)
                nc.scalar.activation(eb3[:], tf3[:], ACTF.Exp)
                em3 = sp.tile([P, NCAT], dt.bfloat16, tag="em3", bufs=2)
                nc.vector.tensor_mul(em3[:], eb3[:], cpT_sb[:])
                cpn_ps = pp.tile([P, NCAT], dt.float32, tag="acc1")
                nc.tensor.matmul(cpn_ps[:], state['pri_norm'][:], em3[:],
                                 start=True, stop=True)
                cpdS_ps = pp.tile([1, NCAT], dt.float32, tag="d1")
                nc.tensor.matmul(cpdS_ps[:], ones_c[:], em3[:], start=True, stop=True)
                cpdZ_ps = pp.tile([1, NCAT], dt.float32, tag="d2")
                nc.tensor.matmul(cpdZ_ps[:], ones_c[:], eb3[:], start=True, stop=True)
                dcp = sp.tile([1, NCAT], dt.float32, tag="dcp", bufs=2)
                nc.vector.tensor_scalar(dcp[:], cpdZ_ps[:], 1e-8, None, op0=ALU.mult)
                nc.vector.tensor_add(dcp[:], dcp[:], cpdS_ps[:])
                nc.vector.reciprocal(dcp[:], dcp[:])
                bc4_ps = pp.tile([P, NCAT], dt.float32, tag="d1")
                nc.tensor.matmul(bc4_ps[:], ones_r[:], dcp[:], start=True, stop=True)
                cpn_sb = sp.tile([P, NCAT], dt.float32, tag="cpn_sb", bufs=2)
                nc.scalar.copy(cpn_sb[:], cpn_ps[:])
                e2cT32 = sp.tile([P, NCAT], dt.float32, tag="e2cT32", bufs=2)
                nc.vector.tensor_mul(e2cT32[:], cpn_sb[:], bc4_ps[:])
                e2cT16 = sp.tile([P, NCAT], dt.bfloat16, tag="e2cT16", bufs=2)
                nc.scalar.copy(e2cT16[:], e2cT32[:])

                # cate gate
                cg_ps = pp.tile([P, NCAT], dt.float32, tag="acc2")
                rhs5c = [state['cateT16'][:], e2cT16[:], e3cT16[:], e2cT16[:], e3cT16[:]]
                for i5 in range(5):
                    nc.tensor.matmul(cg_ps[:], W_sb[:, 10 + i5, :], rhs5c[i5],
                                     start=(i5 == 0), stop=(i5 == 4))
                gc_sb = sp.tile([P, NCAT], dt.float32, tag="gc_sb", bufs=2)
                nc.scalar.activation(gc_sb[:], cg_ps[:], ACTF.Sigmoid, bias=bcomb[:, 2:3])
                xc = sp.tile([P, NCAT], dt.float32, tag="xc", bufs=2)
                nc.vector.tensor_sub(xc[:], e2cT32[:], e3cT32[:])
                nc.vector.tensor_mul(xc[:], xc[:], gc_sb[:])
                nc.vector.tensor_add(xc[:], xc[:], state['cateT32'][:])
                nc.vector.tensor_add(xc[:], xc[:], e3cT32[:])
                cateT32n = cp_.tile([P, NCAT], dt.float32, tag=f"cateT32_{L % 2 + 1}")
                nc.vector.tensor_copy(cateT32n[:], xc[:])
                cateT16n = cp_.tile([P, NCAT], dt.bfloat16, tag=f"cateT16_{L % 2 + 1}")
                nc.scalar.copy(cateT16n[:], xc[:])
                cate_normN = cp_.tile([P, NQC, EMB], dt.bfloat16, tag=f"cate_norm_{L % 2 + 1}")
                for q in range(NQC):
                    lo_q = q * P
                    hi_q = min((q + 1) * P, NCAT)
                    n_q = hi_q - lo_q
                    cn_ps = pp.tile([P, P], dt.bfloat16, tag="d2")
                    nc.tensor.transpose(cn_ps[0:n_q, :], cateT16n[:, lo_q:hi_q], ident[:])
                    if n_q < P:
                        nc.vector.memset(cate_normN[:, q, :], 0.0)
                    nc.vector.tensor_copy(cate_normN[0:n_q, q, :], cn_ps[0:n_q, :])

                state = dict(e1d32=itemT32_d[L], e1d16=itemT16_d[L],
                             normd=ag_in[L],
                             priT32=priT32n, priT16=priT16n, pri_norm=pri_normN,
                             cateT32=cateT32n, cateT16=cateT16n, cate_norm=cate_normN,
                             tab=ag_out[L])

    nc.compile()
    return nc


# ---------------------------------------------------------------------------

_CACHE = {}


def run_on_hw(inputs, cfg=None, trace=False):
    from concourse import bass_utils
    cfg = cfg or Cfg()
    in_maps, meta = prep(inputs, cfg)
    nc = build(cfg, meta)
    res = bass_utils.run_bass_kernel_spmd(
        nc, in_maps, core_ids=list(range(cfg.NCORE)), trace=trace)
    return res, meta, cfg


def _assemble(results, meta, cfg):
    item = np.concatenate(
        [np.asarray(results[co]['item_t'], np.float32).T for co in range(cfg.NCORE)],
        axis=0)
    price_perm = np.asarray(results[0]['price_t'], np.float32)[:, :cfg.NP].T
    price = np.empty_like(price_perm)
    price[meta['perm_p']] = price_perm
    return item, price


def kernel(**inputs):
    res, meta, cfg = run_on_hw(inputs)
    return _assemble(res.results, meta, cfg)
